# revision 4
# baseline (speedup 1.0000x reference)
"""Trainium2 Bass kernel for nn_PlainDecoder (2-layer 2-dir GRU decoder +
vocab projection + log_softmax).

Sharding: data-parallel over batch (4 per core) for the scan; vocab-parallel
(4096-wide shard of padded 32768) for the logits.

Scan design (transposed orientation): all gate matmuls output
[128 gate-partitions, 4 batch] so the PE bill (prop. to output FREE size) is
tiny and h' is produced directly in lhsT (hidden-major) layout -- no PE
transposes.  Per (layer, dir) a windowed PSUM tile P holds, per step, 16
slots of 128 gates: [0:8]=r|z (preloaded with gi+bias), [8:12]=n-gh
(preloaded with b_hh_n), [12:16]=gi_n (+b_ih_n).  A bias matmul opens each
window bank (start=True), the windowed gi GEMM and the per-step Whh matmuls
accumulate on top (start=False).  Both layers run interleaved (layer 1 lags
LAG steps).  Everything bf16 into the PE, f32 in PSUM.

Logits: x2 (= layer-1 hist, bf16) is scaled to fp8 and AllGather'd; fc_w is
fp8.  Matmuls run in DoubleRow perf mode (K=256/instr, 0.5 cyc/row).  Per
(128-row block, 1024-vocab chunk): exp(l/256) with accumulated row sums and
a bf16 copy of l (frees PSUM fast); one AllReduce of partial sums per block;
out = l - ln(S) written bf16 (host converts to f32).
"""

import os
import sys
from contextlib import ExitStack

for _p in ("/opt/trn_rl_repo", "/root/.axon_site/_ro/trn_rl_repo"):
    if os.path.isdir(_p) and _p not in sys.path:
        sys.path.insert(0, _p)

import numpy as np  # noqa: E402
import ml_dtypes  # noqa: E402

V, E, H, L, B, S = 32000, 512, 512, 2, 32, 128
NC_ = 8                      # cores
BPC = B // NC_               # batches per core = 4
R = BPC * S                  # rows per core = 512 (s-major: row = 4*t + b)
VPAD = 32768
VS = VPAD // NC_             # vocab shard per core = 4096
W = 8                        # scan PSUM window (steps)
LAG = 12                     # layer-1 lag (steps)
NW = S // W                  # 16 windows
SW = 64.0                    # fc_w fp8 scale
SX = 4.0                     # x2 fp8 scale
SREC = 1.0 / (SW * SX)       # logits descale
PADB = -240.0                # pad-vocab scaled bias (e4m3 max finite)
NROW = NC_ * R               # 4096 global rows
NBLK = NROW // 128           # 32 row blocks

_BUILT = {}


def _build_nc(n_cores=NC_, sim=False):
    """Build the Bass program (same NEFF for all cores; per-core data
    differs).  sim=True replaces collectives with local DMAs so TimelineSim
    can run."""
    import concourse.bass as bass  # noqa: F401
    import concourse.mybir as mybir
    import concourse.tile as tile
    from concourse import bacc

    dt = mybir.dt
    f32 = dt.float32
    bf = dt.bfloat16
    f8 = dt.float8e4
    AF = mybir.ActivationFunctionType
    OP = mybir.AluOpType
    PM = mybir.MatmulPerfMode

    nc = bacc.Bacc("TRN2", target_bir_lowering=False, debug=False,
                   num_devices=n_cores)

    # ---------------- DRAM I/O ----------------
    embT = nc.dram_tensor("embT", [128, 4, R], bf, kind="ExternalInput")
    h0T = nc.dram_tensor("h0T", [128, 2, 2, 4, BPC], bf, kind="ExternalInput")
    WihT0 = nc.dram_tensor("WihT0", [128, 4, 2, 12, 128], bf,
                           kind="ExternalInput")
    WhhT0 = nc.dram_tensor("WhhT0", [128, 4, 2, 12, 128], bf,
                           kind="ExternalInput")
    WihT1 = nc.dram_tensor("WihT1", [128, 8, 2, 12, 128], bf,
                           kind="ExternalInput")
    WhhT1 = nc.dram_tensor("WhhT1", [128, 4, 2, 12, 128], bf,
                           kind="ExternalInput")
    biasT = nc.dram_tensor("biasT", [16, 2, 2, 128], bf, kind="ExternalInput")
    sel16 = nc.dram_tensor("sel16", [16, 16, W, BPC], bf, kind="ExternalInput")
    fcw8 = nc.dram_tensor("fcw8", [128, 8, VS], f8, kind="ExternalInput")
    fcb8 = nc.dram_tensor("fcb8", [1, 4, 2, 1024], f8, kind="ExternalInput")
    ones8 = nc.dram_tensor("ones8", [1, 2, 128], f8, kind="ExternalInput")

    out_d = nc.dram_tensor("out", [NROW, VS], bf, kind="ExternalOutput")

    # internal DRAM for collectives
    agx_in = nc.dram_tensor("agx_in", [128, 8, R], f8, kind="Internal")
    agx_out = nc.dram_tensor("agx_out", [n_cores * 128, 8, R], f8,
                             kind="Internal", addr_space="Shared")
    ags_in = [nc.dram_tensor(f"ags_in{g}", [128, 1], f32, kind="Internal")
              for g in range(NBLK)]
    ags_out = [nc.dram_tensor(f"ags_out{g}", [128, 1], f32,
                              kind="Internal", addr_space="Shared")
               for g in range(NBLK)]
    rg = [list(range(n_cores))]

    with tile.TileContext(nc) as tc, ExitStack() as top:
        # ---------------- scan phase ----------------
        with ExitStack() as scan_stack:
            wpool = scan_stack.enter_context(tc.tile_pool(name="wts", bufs=1))
            hpool = scan_stack.enter_context(tc.tile_pool(name="hist", bufs=1))
            cpool = scan_stack.enter_context(tc.tile_pool(name="chain", bufs=3))
            p0pool = scan_stack.enter_context(
                tc.tile_pool(name="P0", bufs=2, space="PSUM"))
            p1pool = scan_stack.enter_context(
                tc.tile_pool(name="P1", bufs=2, space="PSUM"))

            embT_sb = wpool.tile([128, 4, R], bf, tag="embT", name="embT_sb")
            nc.sync.dma_start(embT_sb[:], embT[:])
            h0_sb = wpool.tile([128, 2, 2, 4, BPC], bf, tag="h0", name="h0_sb")
            nc.sync.dma_start(h0_sb[:], h0T[:])
            wih0 = wpool.tile([128, 4, 2, 12, 128], bf, tag="wih0",
                              name="wih0")
            nc.sync.dma_start(wih0[:], WihT0[:])
            whh0 = wpool.tile([128, 4, 2, 12, 128], bf, tag="whh0",
                              name="whh0")
            nc.sync.dma_start(whh0[:], WhhT0[:])
            wih1 = wpool.tile([128, 8, 2, 12, 128], bf, tag="wih1",
                              name="wih1")
            nc.sync.dma_start(wih1[:], WihT1[:])
            whh1 = wpool.tile([128, 4, 2, 12, 128], bf, tag="whh1",
                              name="whh1")
            nc.sync.dma_start(whh1[:], WhhT1[:])
            bias_sb = wpool.tile([16, 2, 2, 128], bf, tag="bias",
                                 name="bias_sb")
            nc.sync.dma_start(bias_sb[:], biasT[:])
            sel_sb = wpool.tile([16, 16, W, BPC], bf, tag="sel", name="sel_sb")
            nc.sync.dma_start(sel_sb[:], sel16[:])

            # hist layout: [128 h-part, dir, kchunk, row(=4t+b)]
            hist = [hpool.tile([128, 2, 4, R], bf, tag=f"hist{l}",
                               name=f"hist{l}") for l in range(2)]
            ppool = [p0pool, p1pool]
            wih = [wih0, wih1]
            whh = [whh0, whh1]
            kc_ih = [4, 8]
            pwin = [{}, {}]       # (layer, window) -> PSUM tile

            def gi_window(l, w):
                """Bias opener + gi GEMM for window w of layer l.
                P layout: [128, dir, slot16, W, BPC]."""
                P = ppool[l].tile([128, 2, 16, W, BPC], f32, tag=f"P{l}",
                                  name=f"P{l}w{w}")
                pwin[l][w] = P
                rows = slice(BPC * W * w, BPC * W * (w + 1))
                for d in range(2):
                    nc.tensor.matmul(P[:, d], bias_sb[:, l, d, :], sel_sb[:],
                                     start=True, stop=False,
                                     skip_group_check=True)
                for d in range(2):
                    for sl in range(12):
                        slot = sl if sl < 8 else sl + 4
                        for k in range(kc_ih[l]):
                            if l == 0:
                                rhs = embT_sb[:, k, rows]
                            else:
                                rhs = hist[0][:, k // 4, k % 4, rows]
                            nc.tensor.matmul(
                                P[:, d, slot], wih[l][:, k, d, sl, :], rhs,
                                start=False,
                                stop=(slot >= 12 and k == kc_ih[l] - 1),
                                skip_group_check=True)

            def scan_step(l, t):
                """Whh matmuls + GRU cell chain for step t of layer l."""
                P = pwin[l][t // W]
                tw = t % W
                if t == 0:
                    hp = h0_sb[:, l]                      # [128, 2, 4, BPC]
                else:
                    hp = hist[l][:, :, :, BPC * (t - 1):BPC * t]
                for d in range(2):
                    for j in range(12):
                        for k in range(4):
                            nc.tensor.matmul(
                                P[:, d, j, tw, :], whh[l][:, k, d, j, :],
                                hp[:, d, k, :], start=False,
                                stop=(k == 3), skip_group_check=True)
                # GRU cell chain (both dirs in one op each)
                rzs = cpool.tile([128, 2, 8, BPC], f32, tag=f"rzs{l}",
                                 name=f"rzs{l}")
                nc.scalar.activation(rzs[:], P[:, :, 0:8, tw, :], AF.Sigmoid)
                n1 = cpool.tile([128, 2, 4, BPC], f32, tag=f"n1{l}",
                                name=f"n1{l}")
                nc.vector.tensor_mul(n1[:], P[:, :, 8:12, tw, :],
                                     rzs[:, :, 0:4, :])
                nc.vector.tensor_add(n1[:], n1[:], P[:, :, 12:16, tw, :])
                nt = cpool.tile([128, 2, 4, BPC], f32, tag=f"nt{l}",
                                name=f"nt{l}")
                nc.scalar.activation(nt[:], n1[:], AF.Tanh)
                # h' = nt + z*(hprev - nt)  (SBUF operands only)
                d1 = cpool.tile([128, 2, 4, BPC], f32, tag=f"d1{l}",
                                name=f"d1{l}")
                eng = nc.vector if l == 0 else nc.gpsimd
                eng.tensor_sub(d1[:], hp[:], nt[:])
                eng.tensor_mul(d1[:], d1[:], rzs[:, :, 4:8, :])
                eng.tensor_add(hist[l][:, :, :, BPC * t:BPC * (t + 1)],
                               d1[:], nt[:])

            gi_window(0, 0)
            for it in range(S + LAG):
                if it % W == 5 and (it + 3) // W < NW:
                    gi_window(0, (it + 3) // W)
                if it % W == 1 and it >= 9 and (it - 9) // W < NW:
                    gi_window(1, (it - 9) // W)
                if it < S:
                    scan_step(0, it)
                t1 = it - LAG
                if 0 <= t1 < S:
                    scan_step(1, t1)

            # x2 -> fp8, ship out for the AllGather
            x8 = cpool.tile([128, 2, 4, R], f8, tag="x8", bufs=1, name="x8")
            nc.vector.tensor_scalar_mul(x8[:], hist[1][:], SX)
            nc.sync.dma_start(agx_in[:],
                              x8[:].rearrange("p d k r -> p (d k) r"))

        if sim:
            nc.sync.dma_start(agx_out[0:128], agx_in[:])
        else:
            nc.gpsimd.collective_compute(
                "AllGather", OP.bypass, replica_groups=rg,
                ins=[agx_in[:].opt()], outs=[agx_out[:].opt()])

        # ---------------- logits + log_softmax ----------------
        with (
            tc.tile_pool(name="fw", bufs=1) as fwpool,
            tc.tile_pool(name="lt", bufs=3) as ltpool,
            tc.tile_pool(name="lps", bufs=4, space="PSUM") as lpspool,
            tc.tile_pool(name="lsc", bufs=2) as lscpool,
        ):
            fw = fwpool.tile([128, 8, VS], f8, tag="fw", name="fw")
            nc.sync.dma_start(fw[:], fcw8[:])
            fb = fwpool.tile([1, 4, 2, 1024], f8, tag="fb", name="fb")
            nc.sync.dma_start(fb[:], fcb8[:])
            on8 = fwpool.tile([1, 2, 128], f8, tag="on8", name="on8")
            nc.sync.dma_start(on8[:], ones8[:])
            x2g = fwpool.tile([128, 8, 8, R], f8, tag="x2g", name="x2g")
            nc.sync.dma_start(
                x2g[:], agx_out[:].rearrange("(c p) k r -> p k c r", p=128))

            prev_out = [None]

            def block(rb):
                csrc, r0 = rb // BPC, (rb % BPC) * 128
                lb = ltpool.tile([128, 4, 1024], bf, tag="lb", name="lb")
                ob = ltpool.tile([128, VS], bf, tag="ob", name="ob")
                srb = lscpool.tile([128, 4], f32, tag="srb", name="srb")
                for vq in range(4):
                    P = lpspool.tile([128, 1024], f32, tag="lp", name="lp")
                    # matmuls per 512-wide half: a PSUM matmul target must
                    # stay within one 2KB bank
                    for hh in range(2):
                        v0 = 1024 * vq + 512 * hh
                        nc.tensor.matmul(P[:, 512 * hh:512 * (hh + 1)],
                                         on8[:], fb[0:1, vq, :,
                                                    512 * hh:512 * (hh + 1)],
                                         start=True, stop=False,
                                         perf_mode=PM.DoubleRow,
                                         skip_group_check=True)
                        for c2 in range(4):
                            nc.tensor.matmul(
                                P[:, 512 * hh:512 * (hh + 1)],
                                x2g[:, 2 * c2:2 * c2 + 2, csrc, r0:r0 + 128],
                                fw[:, 2 * c2:2 * c2 + 2, v0:v0 + 512],
                                start=False, stop=(c2 == 3),
                                perf_mode=PM.DoubleRow, skip_group_check=True)
                    eb = lscpool.tile([128, 1024], bf, tag="eb", name="eb")
                    nc.scalar.activation(eb[:], P[:], AF.Exp, scale=SREC,
                                         accum_out=srb[:, vq:vq + 1])
                    # bf16 copy of l (frees PSUM): vq0 on ACT, rest on DVE
                    if vq == 0:
                        nc.scalar.mul(lb[:, vq, :], P[:], SREC)
                    else:
                        nc.vector.tensor_scalar_mul(lb[:, vq, :], P[:], SREC)
                s1 = lscpool.tile([128, 1], f32, tag="s1", name="s1")
                nc.vector.tensor_reduce(s1[:], srb[:],
                                        axis=mybir.AxisListType.X, op=OP.add)
                # write previous block's output before touching SP with the
                # collective chain for this block
                if prev_out[0] is not None:
                    prb, pob = prev_out[0]
                    nc.sync.dma_start(out_d[128 * prb:128 * (prb + 1), :],
                                      pob[:])
                nc.sync.dma_start(ags_in[rb][:], s1[:])
                if sim:
                    nc.sync.dma_start(ags_out[rb][:], ags_in[rb][:])
                else:
                    nc.gpsimd.collective_compute(
                        "AllReduce", OP.add, replica_groups=rg,
                        ins=[ags_in[rb][:].opt()],
                        outs=[ags_out[rb][:].opt()])
                sg = lscpool.tile([128, 1], f32, tag="sg", name="sg")
                nc.sync.dma_start(sg[:], ags_out[rb][:])
                lnS = lscpool.tile([128, 1], f32, tag="lnS", name="lnS")
                nc.scalar.activation(lnS[:], sg[:], AF.Ln)
                for vq in range(4):
                    eng = nc.vector if vq < 2 else nc.gpsimd
                    eng.tensor_scalar_sub(ob[:, 1024 * vq:1024 * (vq + 1)],
                                          lb[:, vq, :], lnS[:])
                prev_out[0] = (rb, ob)

            for rb in range(NBLK):
                block(rb)
            prb, pob = prev_out[0]
            nc.sync.dma_start(out_d[128 * prb:128 * (prb + 1), :], pob[:])

    nc.compile()
    return nc


def _get_nc():
    if "nc" not in _BUILT:
        _BUILT["nc"] = _build_nc()
    return _BUILT["nc"]


def _prep_inputs(inputs):
    """Host-side shard + relayout. Returns in_maps for 8 cores."""
    bft = ml_dtypes.bfloat16
    f8t = ml_dtypes.float8_e4m3

    tgt = np.asarray(inputs["target"])
    ctx = np.asarray(inputs["context"], np.float32)
    emb_t = np.asarray(inputs["embed_table"], np.float32)
    fc_w = np.asarray(inputs["fc_w"], np.float32)
    fc_b = np.asarray(inputs["fc_b"], np.float32)

    def wT(w, kc):     # [2, 1536, IN] -> [128, kc, 2, 12, 128]
        w = np.asarray(w, np.float32)
        a = w.transpose(2, 0, 1).reshape(kc, 128, 2, 12, 128)
        return np.ascontiguousarray(a.transpose(1, 0, 2, 3, 4)).astype(bft)

    WihT0 = wT(inputs["w_ih0"], 4)
    WhhT0 = wT(inputs["w_hh0"], 4)
    WihT1 = wT(inputs["w_ih1"], 8)
    WhhT1 = wT(inputs["w_hh1"], 4)

    # biasT[slot, layer, dir, g]
    biasT = np.zeros((16, 2, 2, 128), np.float32)
    for l, (bi, bh) in enumerate([
            (np.asarray(inputs["b_ih0"], np.float32),
             np.asarray(inputs["b_hh0"], np.float32)),
            (np.asarray(inputs["b_ih1"], np.float32),
             np.asarray(inputs["b_hh1"], np.float32))]):
        for d in range(2):
            rz = (bi[d, :1024] + bh[d, :1024]).reshape(8, 128)
            biasT[0:8, l, d, :] = rz
            biasT[8:12, l, d, :] = bh[d, 1024:].reshape(4, 128)
            biasT[12:16, l, d, :] = bi[d, 1024:].reshape(4, 128)
    biasT = biasT.astype(bft)

    sel = np.zeros((16, 16, W, BPC), np.float32)
    for s in range(16):
        sel[s, s] = 1.0
    sel = sel.astype(bft)

    fcw_pad = np.zeros((VPAD, 2 * H), np.float32)
    fcw_pad[:V] = fc_w
    fcb_pad = np.full((VPAD,), PADB, np.float32)
    fcb_pad[:V] = fc_b * (SW * SX)

    ones8 = np.zeros((1, 2, 128), np.float32)
    ones8[0, 0, :] = 1.0
    ones8 = ones8.astype(f8t)

    emb = emb_t[tgt]                      # [B, S, E]
    ctx4 = ctx.reshape(L, 2, B, H)        # [l, d, b, h]

    in_maps = []
    for c in range(NC_):
        bs = slice(BPC * c, BPC * (c + 1))
        emb_rows = emb[bs].transpose(1, 0, 2).reshape(R, E)   # row = 4t+b
        embT = np.ascontiguousarray(
            emb_rows.T.reshape(4, 128, R).transpose(1, 0, 2)).astype(bft)
        cc = ctx4[:, :, bs, :]                                # [l, d, 4, h]
        h0a = cc.transpose(3, 0, 1, 2).reshape(4, 128, L, 2, BPC)
        h0T = np.ascontiguousarray(
            h0a.transpose(1, 2, 3, 0, 4)).astype(bft)
        shard = fcw_pad[VS * c:VS * (c + 1)] * SW             # [VS, 1024]
        fcw8 = np.ascontiguousarray(
            shard.T.reshape(8, 128, VS).transpose(1, 0, 2)).astype(f8t)
        fcb8 = np.zeros((1, 4, 2, 1024), np.float32)
        fcb8[0, :, 0, :] = fcb_pad[VS * c:VS * (c + 1)].reshape(4, 1024)
        fcb8 = fcb8.astype(f8t)
        in_maps.append({
            "embT": embT, "h0T": h0T,
            "WihT0": WihT0, "WhhT0": WhhT0,
            "WihT1": WihT1, "WhhT1": WhhT1,
            "biasT": biasT, "sel16": sel,
            "fcw8": fcw8, "fcb8": fcb8, "ones8": ones8,
        })
    return in_maps


def _unshard(results):
    Lfull = np.concatenate(
        [results[c]["out"].astype(np.float32) for c in range(NC_)], axis=1)
    Lfull = Lfull[:, :V]                  # [4096, 32000]
    b = np.arange(B)[:, None]
    s = np.arange(S)[None, :]
    rows = (b // BPC) * R + BPC * s + (b % BPC)
    return Lfull[rows]                    # [B, S, V]


def kernel(**inputs):
    from concourse.bass_utils import run_bass_kernel_spmd
    nc = _get_nc()
    in_maps = _prep_inputs(inputs)
    res = run_bass_kernel_spmd(nc, in_maps, core_ids=list(range(NC_)))
    return _unshard(res.results)


# revision 9
# speedup vs baseline: 1.0720x; 1.0720x over previous
"""Trainium2 Bass kernel for nn_PlainDecoder (2-layer 2-dir GRU decoder +
vocab projection + log_softmax).

Sharding: data-parallel over batch (4 per core) for the scan; vocab-parallel
(4096-wide shard of padded 32768) for the logits.

Scan design (transposed orientation): all gate matmuls output
[128 gate-partitions, 4 batch] so the PE bill (prop. to output FREE size) is
tiny and h' is produced directly in lhsT (hidden-major) layout -- no PE
transposes.  Per (layer, dir) a windowed PSUM tile P holds, per step, 16
slots of 128 gates: [0:8]=r|z (preloaded with gi+bias), [8:12]=n-gh
(preloaded with b_hh_n), [12:16]=gi_n (+b_ih_n).  A bias matmul opens each
window bank (start=True), the windowed gi GEMM and the per-step Whh matmuls
accumulate on top (start=False).  Both layers run interleaved (layer 1 lags
LAG steps).  Everything bf16 into the PE, f32 in PSUM.

Logits: x2 (= layer-1 hist, bf16) is scaled to fp8 and AllGather'd; fc_w is
fp8.  Matmuls run in DoubleRow perf mode (K=256/instr, 0.5 cyc/row).  Per
(128-row block, 1024-vocab chunk): exp(l/256) with accumulated row sums and
a bf16 copy of l (frees PSUM fast); one AllReduce of partial sums per block;
out = l - ln(S) written bf16 (host converts to f32).
"""

import os
import sys
from contextlib import ExitStack

for _p in ("/opt/trn_rl_repo", "/root/.axon_site/_ro/trn_rl_repo"):
    if os.path.isdir(_p) and _p not in sys.path:
        sys.path.insert(0, _p)

import numpy as np  # noqa: E402
import ml_dtypes  # noqa: E402

V, E, H, L, B, S = 32000, 512, 512, 2, 32, 128
NC_ = 8                      # cores
BPC = B // NC_               # batches per core = 4
R = BPC * S                  # rows per core = 512 (s-major: row = 4*t + b)
VPAD = 32768
VS = VPAD // NC_             # vocab shard per core = 4096
W = 8                        # scan PSUM window (steps)
LAG = 12                     # layer-1 lag (steps)
NW = S // W                  # 16 windows
SW = 64.0                    # fc_w fp8 scale
SX = 4.0                     # x2 fp8 scale
SREC = 1.0 / (SW * SX)       # logits descale
PADB = -240.0                # pad-vocab scaled bias (e4m3 max finite)
NROW = NC_ * R               # 4096 global rows
NBLK = NROW // 128           # 32 row blocks

_BUILT = {}


def _build_nc(n_cores=NC_, sim=False):
    """Build the Bass program (same NEFF for all cores; per-core data
    differs).  sim=True replaces collectives with local DMAs so TimelineSim
    can run."""
    import concourse.bass as bass  # noqa: F401
    import concourse.mybir as mybir
    import concourse.tile as tile
    from concourse import bacc

    dt = mybir.dt
    f32 = dt.float32
    bf = dt.bfloat16
    f8 = dt.float8e4
    AF = mybir.ActivationFunctionType
    OP = mybir.AluOpType
    PM = mybir.MatmulPerfMode

    nc = bacc.Bacc("TRN2", target_bir_lowering=False, debug=False,
                   num_devices=n_cores)

    # ---------------- DRAM I/O ----------------
    embT = nc.dram_tensor("embT", [128, 4, R], bf, kind="ExternalInput")
    h0T = nc.dram_tensor("h0T", [128, 2, 2, 4, BPC], bf, kind="ExternalInput")
    WihT0 = nc.dram_tensor("WihT0", [128, 4, 2, 12, 128], bf,
                           kind="ExternalInput")
    WhhT0 = nc.dram_tensor("WhhT0", [128, 4, 2, 12, 128], bf,
                           kind="ExternalInput")
    WihT1 = nc.dram_tensor("WihT1", [128, 8, 2, 12, 128], bf,
                           kind="ExternalInput")
    WhhT1 = nc.dram_tensor("WhhT1", [128, 4, 2, 12, 128], bf,
                           kind="ExternalInput")
    biasT = nc.dram_tensor("biasT", [16, 2, 2, 128], bf, kind="ExternalInput")
    sel16 = nc.dram_tensor("sel16", [16, 16, W, BPC], bf, kind="ExternalInput")
    fcw8 = nc.dram_tensor("fcw8", [128, 8, VS], f8, kind="ExternalInput")
    fcb8 = nc.dram_tensor("fcb8", [1, 4, 2, 1024], f8, kind="ExternalInput")
    ones8 = nc.dram_tensor("ones8", [1, 2, 128], f8, kind="ExternalInput")

    out_d = nc.dram_tensor("out", [NROW, VS], bf, kind="ExternalOutput")

    # internal DRAM for collectives
    agx_in = nc.dram_tensor("agx_in", [128, 8, R], f8, kind="Internal")
    agx_out = nc.dram_tensor("agx_out", [n_cores * 128, 8, R], f8,
                             kind="Internal", addr_space="Shared")
    NGRP = NBLK // 4          # 8 sum-collective groups of 4 row blocks
    ags_in = [nc.dram_tensor(f"ags_in{g}", [128, 4], f32, kind="Internal")
              for g in range(NGRP)]
    ags_out = [nc.dram_tensor(f"ags_out{g}", [128, 4], f32,
                              kind="Internal", addr_space="Shared")
               for g in range(NGRP)]
    rg = [list(range(n_cores))]

    with tile.TileContext(nc) as tc, ExitStack() as top:
        # ---------------- scan phase ----------------
        with ExitStack() as scan_stack:
            wpool = scan_stack.enter_context(tc.tile_pool(name="wts", bufs=1))
            hpool = scan_stack.enter_context(tc.tile_pool(name="hist", bufs=1))
            cpool = scan_stack.enter_context(tc.tile_pool(name="chain", bufs=3))
            p0pool = scan_stack.enter_context(
                tc.tile_pool(name="P0", bufs=2, space="PSUM"))
            p1pool = scan_stack.enter_context(
                tc.tile_pool(name="P1", bufs=2, space="PSUM"))

            embT_sb = wpool.tile([128, 4, R], bf, tag="embT", name="embT_sb")
            nc.sync.dma_start(embT_sb[:], embT[:])
            h0_sb = wpool.tile([128, 2, 2, 4, BPC], bf, tag="h0", name="h0_sb")
            nc.sync.dma_start(h0_sb[:], h0T[:])
            wih0 = wpool.tile([128, 4, 2, 12, 128], bf, tag="wih0",
                              name="wih0")
            nc.sync.dma_start(wih0[:], WihT0[:])
            whh0 = wpool.tile([128, 4, 2, 12, 128], bf, tag="whh0",
                              name="whh0")
            nc.sync.dma_start(whh0[:], WhhT0[:])
            wih1 = wpool.tile([128, 8, 2, 12, 128], bf, tag="wih1",
                              name="wih1")
            nc.sync.dma_start(wih1[:], WihT1[:])
            whh1 = wpool.tile([128, 4, 2, 12, 128], bf, tag="whh1",
                              name="whh1")
            nc.sync.dma_start(whh1[:], WhhT1[:])
            bias_sb = wpool.tile([16, 2, 2, 128], bf, tag="bias",
                                 name="bias_sb")
            nc.sync.dma_start(bias_sb[:], biasT[:])
            sel_sb = wpool.tile([16, 16, W, BPC], bf, tag="sel", name="sel_sb")
            nc.sync.dma_start(sel_sb[:], sel16[:])

            # hist layout: [128 h-part, dir, kchunk, row(=4t+b)]
            hist = [hpool.tile([128, 2, 4, R], bf, tag=f"hist{l}",
                               name=f"hist{l}") for l in range(2)]
            ppool = [p0pool, p1pool]
            wih = [wih0, wih1]
            whh = [whh0, whh1]
            kc_ih = [4, 8]
            pwin = [{}, {}]       # (layer, window) -> PSUM tile
            gwin = [{}, {}]       # (layer, window) -> SBUF gi_n tile

            def gi_window(l, w):
                """Bias opener + gi GEMM for window w of layer l.
                P layout: [128, dir, slot16, W, BPC]."""
                P = ppool[l].tile([128, 2, 16, W, BPC], f32, tag=f"P{l}",
                                  name=f"P{l}w{w}")
                pwin[l][w] = P
                rows = slice(BPC * W * w, BPC * W * (w + 1))
                for d in range(2):
                    nc.tensor.matmul(P[:, d], bias_sb[:, l, d, :], sel_sb[:],
                                     start=True, stop=False,
                                     skip_group_check=True)
                for d in range(2):
                    for sl in range(12):
                        slot = sl if sl < 8 else sl + 4
                        for k in range(kc_ih[l]):
                            if l == 0:
                                rhs = embT_sb[:, k, rows]
                            else:
                                rhs = hist[0][:, k // 4, k % 4, rows]
                            nc.tensor.matmul(
                                P[:, d, slot], wih[l][:, k, d, sl, :], rhs,
                                start=False,
                                stop=(slot >= 12 and k == kc_ih[l] - 1),
                                skip_group_check=True)
                # stage gi_n into SBUF so the in-loop add is SBUF-only
                gin = cpool.tile([128, 2, 4, W, BPC], f32, tag=f"gin{l}",
                                 bufs=2, name=f"gin{l}")
                nc.vector.tensor_copy(gin[:], P[:, :, 12:16, :, :])
                gwin[l][w] = gin

            def scan_step(l, t):
                """Whh matmuls + GRU cell chain for step t of layer l."""
                P = pwin[l][t // W]
                gin = gwin[l][t // W]
                tw = t % W
                if t == 0:
                    hp = h0_sb[:, l]                      # [128, 2, 4, BPC]
                else:
                    hp = hist[l][:, :, :, BPC * (t - 1):BPC * t]
                # r|z matmuls first: the sigmoid (head of the serial chain)
                # waits only on these; the n-gh matmuls overlap it
                for j0, j1 in ((0, 8), (8, 12)):
                    for d in range(2):
                        for j in range(j0, j1):
                            for k in range(4):
                                nc.tensor.matmul(
                                    P[:, d, j, tw, :], whh[l][:, k, d, j, :],
                                    hp[:, d, k, :], start=False,
                                    stop=(k == 3), skip_group_check=True)
                # GRU cell chain (both dirs in one op each):
                #   h' = n*(1-z) + z*hprev ; q=z*hprev and u=1-z leave the
                #   serial path right after the sigmoid
                rzs = cpool.tile([128, 2, 8, BPC], f32, tag=f"rzs{l}",
                                 name=f"rzs{l}")
                nc.scalar.activation(rzs[:], P[:, :, 0:8, tw, :], AF.Sigmoid)
                n1 = cpool.tile([128, 2, 4, BPC], f32, tag=f"n1{l}",
                                name=f"n1{l}")
                nc.vector.tensor_mul(n1[:], P[:, :, 8:12, tw, :],
                                     rzs[:, :, 0:4, :])
                nc.vector.tensor_add(n1[:], n1[:], gin[:, :, :, tw, :])
                q = cpool.tile([128, 2, 4, BPC], f32, tag=f"q{l}",
                               name=f"q{l}")
                nc.vector.tensor_mul(q[:], rzs[:, :, 4:8, :], hp[:])
                u = cpool.tile([128, 2, 4, BPC], f32, tag=f"u{l}",
                               name=f"u{l}")
                nc.vector.tensor_scalar(u[:], rzs[:, :, 4:8, :], -1.0, 1.0,
                                        OP.mult, OP.add)
                nt = cpool.tile([128, 2, 4, BPC], f32, tag=f"nt{l}",
                                name=f"nt{l}")
                nc.scalar.activation(nt[:], n1[:], AF.Tanh)
                # tail on Pool (no ack latency, SBUF-only operands)
                d1 = cpool.tile([128, 2, 4, BPC], f32, tag=f"d1{l}",
                                name=f"d1{l}")
                nc.gpsimd.tensor_mul(d1[:], nt[:], u[:])
                nc.gpsimd.tensor_add(hist[l][:, :, :, BPC * t:BPC * (t + 1)],
                                     d1[:], q[:])

            gi_window(0, 0)
            for it in range(S + LAG):
                if it % W == 5 and (it + 3) // W < NW:
                    gi_window(0, (it + 3) // W)
                if it % W == 1 and it >= 9 and (it - 9) // W < NW:
                    gi_window(1, (it - 9) // W)
                if it < S:
                    scan_step(0, it)
                t1 = it - LAG
                if 0 <= t1 < S:
                    scan_step(1, t1)

            # x2 -> fp8, ship out for the AllGather
            x8 = cpool.tile([128, 2, 4, R], f8, tag="x8", bufs=1, name="x8")
            nc.vector.tensor_scalar_mul(x8[:], hist[1][:], SX)
            nc.sync.dma_start(agx_in[:],
                              x8[:].rearrange("p d k r -> p (d k) r"))

        if sim:
            nc.sync.dma_start(agx_out[0:128], agx_in[:])
        else:
            nc.gpsimd.collective_compute(
                "AllGather", OP.bypass, replica_groups=rg,
                ins=[agx_in[:].opt()], outs=[agx_out[:].opt()])

        # ---------------- logits + log_softmax ----------------
        with (
            tc.tile_pool(name="fw", bufs=1) as fwpool,
            tc.tile_pool(name="lt", bufs=3) as ltpool,
            tc.tile_pool(name="lps", bufs=4, space="PSUM") as lpspool,
            tc.tile_pool(name="lsc", bufs=2) as lscpool,
        ):
            fw = fwpool.tile([128, 8, VS], f8, tag="fw", name="fw")
            nc.sync.dma_start(fw[:], fcw8[:])
            fb = fwpool.tile([1, 4, 2, 1024], f8, tag="fb", name="fb")
            nc.sync.dma_start(fb[:], fcb8[:])
            on8 = fwpool.tile([1, 2, 128], f8, tag="on8", name="on8")
            nc.sync.dma_start(on8[:], ones8[:])
            x2g = fwpool.tile([128, 8, 8, R], f8, tag="x2g", name="x2g")
            nc.sync.dma_start(
                x2g[:], agx_out[:].rearrange("(c p) k r -> p k c r", p=128))

            def block_mm(rb, lb, sgrp, rb4):
                """Matmuls + exp/sums + bf16-l copy for one 128-row block."""
                csrc, r0 = rb // BPC, (rb % BPC) * 128
                srb = lscpool.tile([128, 4], f32, tag="srb", name="srb")
                for vq in range(4):
                    P = lpspool.tile([128, 1024], f32, tag="lp", name="lp")
                    # matmuls per 512-wide half: a PSUM matmul target must
                    # stay within one 2KB bank
                    for hh in range(2):
                        v0 = 1024 * vq + 512 * hh
                        nc.tensor.matmul(P[:, 512 * hh:512 * (hh + 1)],
                                         on8[:], fb[0:1, vq, :,
                                                    512 * hh:512 * (hh + 1)],
                                         start=True, stop=False,
                                         perf_mode=PM.DoubleRow,
                                         skip_group_check=True)
                        for c2 in range(4):
                            nc.tensor.matmul(
                                P[:, 512 * hh:512 * (hh + 1)],
                                x2g[:, 2 * c2:2 * c2 + 2, csrc, r0:r0 + 128],
                                fw[:, 2 * c2:2 * c2 + 2, v0:v0 + 512],
                                start=False, stop=(c2 == 3),
                                perf_mode=PM.DoubleRow, skip_group_check=True)
                    eb = lscpool.tile([128, 1024], bf, tag="eb", name="eb")
                    nc.scalar.activation(eb[:], P[:], AF.Exp, scale=SREC,
                                         accum_out=srb[:, vq:vq + 1])
                    # bf16 copy of l frees the PSUM bank quickly
                    nc.vector.tensor_scalar_mul(lb[:, vq, :], P[:], SREC)
                nc.vector.tensor_reduce(sgrp[:, rb4:rb4 + 1], srb[:],
                                        axis=mybir.AxisListType.X, op=OP.add)

            for g in range(NGRP):
                lbs = []
                obs = []
                sgrp = lscpool.tile([128, 4], f32, tag="sgrp", name="sgrp")
                for rb4 in range(4):
                    lb = ltpool.tile([128, 4, 1024], bf, tag="lb", bufs=6,
                                     name="lb")
                    block_mm(4 * g + rb4, lb, sgrp, rb4)
                    lbs.append(lb)
                nc.sync.dma_start(ags_in[g][:], sgrp[:])
                if sim:
                    nc.sync.dma_start(ags_out[g][:], ags_in[g][:])
                else:
                    nc.gpsimd.collective_compute(
                        "AllReduce", OP.add, replica_groups=rg,
                        ins=[ags_in[g][:].opt()],
                        outs=[ags_out[g][:].opt()])
                sg = lscpool.tile([128, 4], f32, tag="sg", name="sg")
                nc.sync.dma_start(sg[:], ags_out[g][:])
                lnS = lscpool.tile([128, 4], f32, tag="lnS", name="lnS")
                nc.scalar.activation(lnS[:], sg[:], AF.Ln)
                for rb4 in range(4):
                    ob = ltpool.tile([128, VS], bf, tag="ob", bufs=4,
                                     name="ob")
                    for vq in range(4):
                        nc.gpsimd.tensor_scalar_sub(
                            ob[:, 1024 * vq:1024 * (vq + 1)],
                            lbs[rb4][:, vq, :], lnS[:, rb4:rb4 + 1])
                    obs.append(ob)
                for rb4 in range(4):
                    rb = 4 * g + rb4
                    nc.sync.dma_start(out_d[128 * rb:128 * (rb + 1), :],
                                      obs[rb4][:])

    nc.compile()
    return nc


def _get_nc():
    if "nc" not in _BUILT:
        _BUILT["nc"] = _build_nc()
    return _BUILT["nc"]


def _prep_inputs(inputs):
    """Host-side shard + relayout. Returns in_maps for 8 cores."""
    bft = ml_dtypes.bfloat16
    f8t = ml_dtypes.float8_e4m3

    tgt = np.asarray(inputs["target"])
    ctx = np.asarray(inputs["context"], np.float32)
    emb_t = np.asarray(inputs["embed_table"], np.float32)
    fc_w = np.asarray(inputs["fc_w"], np.float32)
    fc_b = np.asarray(inputs["fc_b"], np.float32)

    def wT(w, kc):     # [2, 1536, IN] -> [128, kc, 2, 12, 128]
        w = np.asarray(w, np.float32)
        a = w.transpose(2, 0, 1).reshape(kc, 128, 2, 12, 128)
        return np.ascontiguousarray(a.transpose(1, 0, 2, 3, 4)).astype(bft)

    WihT0 = wT(inputs["w_ih0"], 4)
    WhhT0 = wT(inputs["w_hh0"], 4)
    WihT1 = wT(inputs["w_ih1"], 8)
    WhhT1 = wT(inputs["w_hh1"], 4)

    # biasT[slot, layer, dir, g]
    biasT = np.zeros((16, 2, 2, 128), np.float32)
    for l, (bi, bh) in enumerate([
            (np.asarray(inputs["b_ih0"], np.float32),
             np.asarray(inputs["b_hh0"], np.float32)),
            (np.asarray(inputs["b_ih1"], np.float32),
             np.asarray(inputs["b_hh1"], np.float32))]):
        for d in range(2):
            rz = (bi[d, :1024] + bh[d, :1024]).reshape(8, 128)
            biasT[0:8, l, d, :] = rz
            biasT[8:12, l, d, :] = bh[d, 1024:].reshape(4, 128)
            biasT[12:16, l, d, :] = bi[d, 1024:].reshape(4, 128)
    biasT = biasT.astype(bft)

    sel = np.zeros((16, 16, W, BPC), np.float32)
    for s in range(16):
        sel[s, s] = 1.0
    sel = sel.astype(bft)

    fcw_pad = np.zeros((VPAD, 2 * H), np.float32)
    fcw_pad[:V] = fc_w
    fcb_pad = np.full((VPAD,), PADB, np.float32)
    fcb_pad[:V] = fc_b * (SW * SX)

    ones8 = np.zeros((1, 2, 128), np.float32)
    ones8[0, 0, :] = 1.0
    ones8 = ones8.astype(f8t)

    emb = emb_t[tgt]                      # [B, S, E]
    ctx4 = ctx.reshape(L, 2, B, H)        # [l, d, b, h]

    in_maps = []
    for c in range(NC_):
        bs = slice(BPC * c, BPC * (c + 1))
        emb_rows = emb[bs].transpose(1, 0, 2).reshape(R, E)   # row = 4t+b
        embT = np.ascontiguousarray(
            emb_rows.T.reshape(4, 128, R).transpose(1, 0, 2)).astype(bft)
        cc = ctx4[:, :, bs, :]                                # [l, d, 4, h]
        h0a = cc.transpose(3, 0, 1, 2).reshape(4, 128, L, 2, BPC)
        h0T = np.ascontiguousarray(
            h0a.transpose(1, 2, 3, 0, 4)).astype(bft)
        shard = fcw_pad[VS * c:VS * (c + 1)] * SW             # [VS, 1024]
        fcw8 = np.ascontiguousarray(
            shard.T.reshape(8, 128, VS).transpose(1, 0, 2)).astype(f8t)
        fcb8 = np.zeros((1, 4, 2, 1024), np.float32)
        fcb8[0, :, 0, :] = fcb_pad[VS * c:VS * (c + 1)].reshape(4, 1024)
        fcb8 = fcb8.astype(f8t)
        in_maps.append({
            "embT": embT, "h0T": h0T,
            "WihT0": WihT0, "WhhT0": WhhT0,
            "WihT1": WihT1, "WhhT1": WhhT1,
            "biasT": biasT, "sel16": sel,
            "fcw8": fcw8, "fcb8": fcb8, "ones8": ones8,
        })
    return in_maps


def _unshard(results):
    Lfull = np.concatenate(
        [results[c]["out"].astype(np.float32) for c in range(NC_)], axis=1)
    Lfull = Lfull[:, :V]                  # [4096, 32000]
    b = np.arange(B)[:, None]
    s = np.arange(S)[None, :]
    rows = (b // BPC) * R + BPC * s + (b % BPC)
    return Lfull[rows]                    # [B, S, V]


def kernel(**inputs):
    from concourse.bass_utils import run_bass_kernel_spmd
    nc = _get_nc()
    in_maps = _prep_inputs(inputs)
    res = run_bass_kernel_spmd(nc, in_maps, core_ids=list(range(NC_)))
    return _unshard(res.results)


# revision 20
# speedup vs baseline: 1.1746x; 1.0957x over previous
"""Trainium2 Bass kernel for nn_PlainDecoder (2-layer 2-dir GRU decoder +
vocab projection + log_softmax).

Sharding: data-parallel over batch (4 per core) for the scan; vocab-parallel
(4096-wide shard of padded 32768) for the logits.

Scan design (transposed orientation): all gate matmuls output
[128 gate-partitions, 4 batch] so the PE bill (prop. to output FREE size) is
tiny and h' is produced directly in lhsT (hidden-major) layout -- no PE
transposes.  Per (layer, dir) a windowed PSUM tile P holds, per step, 16
slots of 128 gates: [0:8]=r|z (preloaded with gi+bias), [8:12]=n-gh
(preloaded with b_hh_n), [12:16]=gi_n (+b_ih_n).  A bias matmul opens each
window bank (start=True), the windowed gi GEMM and the per-step Whh matmuls
accumulate on top (start=False).  Both layers run interleaved (layer 1 lags
LAG steps).  Everything bf16 into the PE, f32 in PSUM.

Logits: x2 (= layer-1 hist, bf16) is scaled to fp8 and AllGather'd; fc_w is
fp8.  Matmuls run in DoubleRow perf mode (K=256/instr, 0.5 cyc/row).  Per
(128-row block, 1024-vocab chunk): exp(l/256) with accumulated row sums and
a bf16 copy of l (frees PSUM fast); one AllReduce of partial sums per block;
out = l - ln(S) written bf16 (host converts to f32).
"""

import os
import sys
from contextlib import ExitStack

for _p in ("/opt/trn_rl_repo", "/root/.axon_site/_ro/trn_rl_repo"):
    if os.path.isdir(_p) and _p not in sys.path:
        sys.path.insert(0, _p)

import numpy as np  # noqa: E402
import ml_dtypes  # noqa: E402

V, E, H, L, B, S = 32000, 512, 512, 2, 32, 128
NC_ = 8                      # cores
BPC = B // NC_               # batches per core = 4
R = BPC * S                  # rows per core = 512 (s-major: row = 4*t + b)
VPAD = 32768
VS = VPAD // NC_             # vocab shard per core = 4096
W = 8                        # scan PSUM window (steps)
LAG = 12                     # layer-1 lag (steps)
NW = S // W                  # 16 windows
SW = 64.0                    # fc_w fp8 scale
SX = 4.0                     # x2 fp8 scale
SREC = 1.0 / (SW * SX)       # logits descale
PADB = -240.0                # pad-vocab scaled bias (e4m3 max finite)
NROW = NC_ * R               # 4096 global rows
NBLK = NROW // 128           # 32 row blocks

_BUILT = {}


def _build_nc(n_cores=NC_, sim=False):
    """Build the Bass program (same NEFF for all cores; per-core data
    differs).  sim=True replaces collectives with local DMAs so TimelineSim
    can run."""
    import concourse.bass as bass  # noqa: F401
    import concourse.mybir as mybir
    import concourse.tile as tile
    from concourse import bacc

    dt = mybir.dt
    f32 = dt.float32
    bf = dt.bfloat16
    f8 = dt.float8e4
    AF = mybir.ActivationFunctionType
    OP = mybir.AluOpType
    PM = mybir.MatmulPerfMode

    nc = bacc.Bacc("TRN2", target_bir_lowering=False, debug=False,
                   num_devices=n_cores)

    # ---------------- DRAM I/O ----------------
    embT = nc.dram_tensor("embT", [128, 4, R], bf, kind="ExternalInput")
    h0T = nc.dram_tensor("h0T", [128, 2, 2, 4, BPC], bf, kind="ExternalInput")
    WihT0 = nc.dram_tensor("WihT0", [128, 4, 2, 12, 128], bf,
                           kind="ExternalInput")
    WhhT0 = nc.dram_tensor("WhhT0", [128, 4, 2, 12, 128], bf,
                           kind="ExternalInput")
    WihT1 = nc.dram_tensor("WihT1", [128, 8, 2, 12, 128], bf,
                           kind="ExternalInput")
    WhhT1 = nc.dram_tensor("WhhT1", [128, 4, 2, 12, 128], bf,
                           kind="ExternalInput")
    biasT = nc.dram_tensor("biasT", [16, 2, 2, 128], bf, kind="ExternalInput")
    sel16 = nc.dram_tensor("sel16", [16, 16, W, BPC], bf, kind="ExternalInput")
    fcw8 = nc.dram_tensor("fcw8", [128, 8, VS], f8, kind="ExternalInput")
    fcb8 = nc.dram_tensor("fcb8", [1, 4, 2, 1024], f8, kind="ExternalInput")
    ones8 = nc.dram_tensor("ones8", [1, 2, 128], f8, kind="ExternalInput")

    out_d = nc.dram_tensor("out", [NROW, VS], bf, kind="ExternalOutput")

    # internal DRAM for collectives
    agx_in = nc.dram_tensor("agx_in", [128, 8, R], f8, kind="Internal")
    agx_out = nc.dram_tensor("agx_out", [n_cores * 128, 8, R], f8,
                             kind="Internal", addr_space="Shared")
    NGRP = NBLK // 4          # 8 sum-collective groups of 4 row blocks
    ags_in = [nc.dram_tensor(f"ags_in{g}", [128, 4], f32, kind="Internal")
              for g in range(NGRP)]
    ags_out = [nc.dram_tensor(f"ags_out{g}", [128, 4], f32,
                              kind="Internal", addr_space="Shared")
               for g in range(NGRP)]
    rg = [list(range(n_cores))]

    with tile.TileContext(nc) as tc, ExitStack() as top:
        # logits weights: loaded up front so the DMAs ride under the scan
        fwpool = top.enter_context(tc.tile_pool(name="fw", bufs=1))
        fw = fwpool.tile([128, 8, VS], f8, tag="fw", name="fw")
        nc.sync.dma_start(fw[:], fcw8[:])
        fb = fwpool.tile([1, 4, 2, 1024], f8, tag="fb", name="fb")
        nc.sync.dma_start(fb[:], fcb8[:])
        on8 = fwpool.tile([1, 2, 128], f8, tag="on8", name="on8")
        nc.sync.dma_start(on8[:], ones8[:])

        # ---------------- scan phase ----------------
        with ExitStack() as scan_stack:
            wpool = scan_stack.enter_context(tc.tile_pool(name="wts", bufs=1))
            hpool = scan_stack.enter_context(tc.tile_pool(name="hist", bufs=1))
            cpool = scan_stack.enter_context(tc.tile_pool(name="chain", bufs=3))
            p0pool = scan_stack.enter_context(
                tc.tile_pool(name="P0", bufs=2, space="PSUM"))
            p1pool = scan_stack.enter_context(
                tc.tile_pool(name="P1", bufs=2, space="PSUM"))

            embT_sb = wpool.tile([128, 4, R], bf, tag="embT", name="embT_sb")
            nc.sync.dma_start(embT_sb[:], embT[:])
            h0_sb = wpool.tile([128, 2, 2, 4, BPC], bf, tag="h0", name="h0_sb")
            nc.sync.dma_start(h0_sb[:], h0T[:])
            wih0 = wpool.tile([128, 4, 2, 12, 128], bf, tag="wih0",
                              name="wih0")
            nc.sync.dma_start(wih0[:], WihT0[:])
            whh0 = wpool.tile([128, 4, 2, 12, 128], bf, tag="whh0",
                              name="whh0")
            nc.sync.dma_start(whh0[:], WhhT0[:])
            wih1 = wpool.tile([128, 8, 2, 12, 128], bf, tag="wih1",
                              name="wih1")
            nc.sync.dma_start(wih1[:], WihT1[:])
            whh1 = wpool.tile([128, 4, 2, 12, 128], bf, tag="whh1",
                              name="whh1")
            nc.sync.dma_start(whh1[:], WhhT1[:])
            bias_sb = wpool.tile([16, 2, 2, 128], bf, tag="bias",
                                 name="bias_sb")
            nc.sync.dma_start(bias_sb[:], biasT[:])
            sel_sb = wpool.tile([16, 16, W, BPC], bf, tag="sel", name="sel_sb")
            nc.sync.dma_start(sel_sb[:], sel16[:])

            # hist layout: [128 h-part, dir, kchunk, row(=4t+b)]
            hist = [hpool.tile([128, 2, 4, R], bf, tag=f"hist{l}",
                               name=f"hist{l}") for l in range(2)]
            ppool = [p0pool, p1pool]
            wih = [wih0, wih1]
            whh = [whh0, whh1]
            kc_ih = [4, 8]
            pwin = [{}, {}]       # (layer, window) -> PSUM tile
            gwin = [{}, {}]       # (layer, window) -> SBUF gi_n tile

            def gi_window(l, w):
                """Bias opener + gi GEMM for window w of layer l.
                P layout: [128, dir, slot16, W, BPC]."""
                P = ppool[l].tile([128, 2, 16, W, BPC], f32, tag=f"P{l}",
                                  name=f"P{l}w{w}")
                pwin[l][w] = P
                rows = slice(BPC * W * w, BPC * W * (w + 1))
                for d in range(2):
                    nc.tensor.matmul(P[:, d], bias_sb[:, l, d, :], sel_sb[:],
                                     start=True, stop=False,
                                     skip_group_check=True)
                for d in range(2):
                    for sl in range(12):
                        slot = sl if sl < 8 else sl + 4
                        for k in range(kc_ih[l]):
                            if l == 0:
                                rhs = embT_sb[:, k, rows]
                            else:
                                rhs = hist[0][:, k // 4, k % 4, rows]
                            nc.tensor.matmul(
                                P[:, d, slot], wih[l][:, k, d, sl, :], rhs,
                                start=False,
                                stop=(slot >= 12 and k == kc_ih[l] - 1),
                                skip_group_check=True)
                # stage gi_n into SBUF so the in-loop add is SBUF-only
                gin = cpool.tile([128, 2, 4, W, BPC], f32, tag=f"gin{l}",
                                 bufs=2, name=f"gin{l}")
                nc.vector.tensor_copy(gin[:], P[:, :, 12:16, :, :])
                gwin[l][w] = gin

            def scan_step(l, t):
                """Whh matmuls + GRU cell chain for step t of layer l."""
                P = pwin[l][t // W]
                gin = gwin[l][t // W]
                tw = t % W
                if t == 0:
                    hp = h0_sb[:, l]                      # [128, 2, 4, BPC]
                else:
                    hp = hist[l][:, :, :, BPC * (t - 1):BPC * t]
                # k-major, r|z slices first: k0/k1 matmuls wait only on the
                # first half of the h' tail, and the sigmoid (head of the
                # serial chain) doesn't wait on the n-gh matmuls
                for j0, j1 in ((0, 8), (8, 12)):
                    for k in range(4):
                        for d in range(2):
                            for j in range(j0, j1):
                                nc.tensor.matmul(
                                    P[:, d, j, tw, :], whh[l][:, k, d, j, :],
                                    hp[:, d, k, :], start=False,
                                    stop=(k == 3), skip_group_check=True)
                # GRU cell chain (both dirs in one op each):
                #   h' = n*(1-z) + z*hprev ; q=z*hprev and u=1-z leave the
                #   serial path right after the sigmoid
                rzs = cpool.tile([128, 2, 8, BPC], f32, tag=f"rzs{l}",
                                 name=f"rzs{l}")
                nc.scalar.activation(rzs[:], P[:, :, 0:8, tw, :], AF.Sigmoid)
                n1 = cpool.tile([128, 2, 4, BPC], f32, tag=f"n1{l}",
                                name=f"n1{l}")
                nc.vector.tensor_mul(n1[:], P[:, :, 8:12, tw, :],
                                     rzs[:, :, 0:4, :])
                nc.vector.tensor_add(n1[:], n1[:], gin[:, :, :, tw, :])
                q = cpool.tile([128, 2, 4, BPC], f32, tag=f"q{l}",
                               name=f"q{l}")
                nc.vector.tensor_mul(q[:], rzs[:, :, 4:8, :], hp[:])
                u = cpool.tile([128, 2, 4, BPC], f32, tag=f"u{l}",
                               name=f"u{l}")
                nc.vector.tensor_scalar(u[:], rzs[:, :, 4:8, :], -1.0, 1.0,
                                        OP.mult, OP.add)
                nt = cpool.tile([128, 2, 4, BPC], f32, tag=f"nt{l}",
                                name=f"nt{l}")
                nc.scalar.activation(nt[:], n1[:], AF.Tanh)
                # tail on Pool (no ack latency, SBUF-only operands), split per
                # k-half so next-step k0/k1 matmuls start before k2/k3 finish
                d1 = cpool.tile([128, 2, 4, BPC], f32, tag=f"d1{l}",
                                name=f"d1{l}")
                for ks in (slice(0, 2), slice(2, 4)):
                    nc.gpsimd.tensor_mul(d1[:, :, ks], nt[:, :, ks],
                                         u[:, :, ks])
                    nc.gpsimd.tensor_add(
                        hist[l][:, :, ks, BPC * t:BPC * (t + 1)],
                        d1[:, :, ks], q[:, :, ks])

            gi_window(0, 0)
            for it in range(S + LAG):
                if it % W == 5 and (it + 3) // W < NW:
                    gi_window(0, (it + 3) // W)
                if it % W == 1 and it >= 9 and (it - 9) // W < NW:
                    gi_window(1, (it - 9) // W)
                if it < S:
                    scan_step(0, it)
                t1 = it - LAG
                if 0 <= t1 < S:
                    scan_step(1, t1)

            # x2 -> fp8, ship out for the AllGather
            x8 = cpool.tile([128, 2, 4, R], f8, tag="x8", bufs=1, name="x8")
            nc.vector.tensor_scalar_mul(x8[:], hist[1][:], SX)
            nc.sync.dma_start(agx_in[:],
                              x8[:].rearrange("p d k r -> p (d k) r"))

        if sim:
            nc.sync.dma_start(agx_out[0:128], agx_in[:])
        else:
            nc.gpsimd.collective_compute(
                "AllGather", OP.bypass, replica_groups=rg,
                ins=[agx_in[:].opt()], outs=[agx_out[:].opt()])

        # ---------------- logits + log_softmax ----------------
        with (
            tc.tile_pool(name="lt", bufs=3) as ltpool,
            tc.tile_pool(name="lps", bufs=4, space="PSUM") as lpspool,
            tc.tile_pool(name="lsc", bufs=2) as lscpool,
        ):
            x2g = ltpool.tile([128, 8, 8, R], f8, tag="x2g", bufs=1,
                              name="x2g")
            # per-source-core chunks: contiguous DMAs, and the first row
            # blocks can start before the later chunks land
            for c in range(8):
                nc.sync.dma_start(x2g[:, :, c, :],
                                  agx_out[128 * c:128 * (c + 1), :, :])

            def block_mm(rb, lb, sgrp, rb4):
                """Matmuls + exp/sums + bf16-l copy for one 128-row block.
                All 40 matmuls are emitted back-to-back (keeps the PE
                p-state ramped); exp/copies follow."""
                csrc, r0 = rb // BPC, (rb % BPC) * 128
                srb = lscpool.tile([128, 4], f32, tag="srb", bufs=3,
                                   name="srb")
                Ps = []
                for vq in range(4):
                    P = lpspool.tile([128, 1024], f32, tag="lp", name="lp")
                    Ps.append(P)
                    # matmuls per 512-wide half: a PSUM matmul target must
                    # stay within one 2KB bank
                    for hh in range(2):
                        v0 = 1024 * vq + 512 * hh
                        nc.tensor.matmul(P[:, 512 * hh:512 * (hh + 1)],
                                         on8[:], fb[0:1, vq, :,
                                                    512 * hh:512 * (hh + 1)],
                                         start=True, stop=False,
                                         perf_mode=PM.DoubleRow,
                                         skip_group_check=True)
                        for c2 in range(4):
                            nc.tensor.matmul(
                                P[:, 512 * hh:512 * (hh + 1)],
                                x2g[:, 2 * c2:2 * c2 + 2, csrc, r0:r0 + 128],
                                fw[:, 2 * c2:2 * c2 + 2, v0:v0 + 512],
                                start=False, stop=(c2 == 3),
                                perf_mode=PM.DoubleRow, skip_group_check=True)
                for vq in range(4):
                    eb = lscpool.tile([128, 1024], bf, tag="eb", bufs=3,
                                      name="eb")
                    nc.scalar.activation(eb[:], Ps[vq][:], AF.Exp, scale=SREC,
                                         accum_out=srb[:, vq:vq + 1])
                    # bf16 copy of l frees the PSUM bank quickly
                    nc.vector.tensor_scalar_mul(lb[:, vq, :], Ps[vq][:], SREC)
                nc.vector.tensor_reduce(sgrp[:, rb4:rb4 + 1], srb[:],
                                        axis=mybir.AxisListType.X, op=OP.add)

            for g in range(NGRP):
                lbs = []
                obs = []
                sgrp = lscpool.tile([128, 4], f32, tag="sgrp", bufs=3,
                                    name="sgrp")
                for rb4 in range(4):
                    lb = ltpool.tile([128, 4, 1024], bf, tag="lb", bufs=6,
                                     name="lb")
                    block_mm(4 * g + rb4, lb, sgrp, rb4)
                    lbs.append(lb)
                nc.sync.dma_start(ags_in[g][:], sgrp[:])
                if sim:
                    nc.sync.dma_start(ags_out[g][:], ags_in[g][:])
                else:
                    nc.gpsimd.collective_compute(
                        "AllReduce", OP.add, replica_groups=rg,
                        ins=[ags_in[g][:].opt()],
                        outs=[ags_out[g][:].opt()])
                sg = lscpool.tile([128, 4], f32, tag="sg", name="sg")
                nc.sync.dma_start(sg[:], ags_out[g][:])
                lnS = lscpool.tile([128, 4], f32, tag="lnS", name="lnS")
                nc.scalar.activation(lnS[:], sg[:], AF.Ln)
                negS = lscpool.tile([128, 4], f32, tag="negS", name="negS")
                nc.vector.tensor_scalar_mul(negS[:], lnS[:], -1.0)
                for rb4 in range(4):
                    ob = ltpool.tile([128, VS], bf, tag="ob", bufs=4,
                                     name="ob")
                    for vq in range(4):
                        # balance the subtract across Pool / DVE / ACT
                        if vq < 2:
                            nc.gpsimd.tensor_scalar_sub(
                                ob[:, 1024 * vq:1024 * (vq + 1)],
                                lbs[rb4][:, vq, :], lnS[:, rb4:rb4 + 1])
                        elif vq == 2:
                            nc.vector.tensor_scalar_sub(
                                ob[:, 1024 * vq:1024 * (vq + 1)],
                                lbs[rb4][:, vq, :], lnS[:, rb4:rb4 + 1])
                        else:
                            nc.scalar.activation(
                                ob[:, 1024 * vq:1024 * (vq + 1)],
                                lbs[rb4][:, vq, :], AF.Identity,
                                bias=negS[:, rb4:rb4 + 1])
                    obs.append(ob)
                for rb4 in range(4):
                    rb = 4 * g + rb4
                    nc.sync.dma_start(out_d[128 * rb:128 * (rb + 1), :],
                                      obs[rb4][:])

    nc.compile()
    return nc


def _get_nc():
    if "nc" not in _BUILT:
        _BUILT["nc"] = _build_nc()
    return _BUILT["nc"]


def _prep_inputs(inputs):
    """Host-side shard + relayout. Returns in_maps for 8 cores."""
    bft = ml_dtypes.bfloat16
    f8t = ml_dtypes.float8_e4m3

    tgt = np.asarray(inputs["target"])
    ctx = np.asarray(inputs["context"], np.float32)
    emb_t = np.asarray(inputs["embed_table"], np.float32)
    fc_w = np.asarray(inputs["fc_w"], np.float32)
    fc_b = np.asarray(inputs["fc_b"], np.float32)

    def wT(w, kc):     # [2, 1536, IN] -> [128, kc, 2, 12, 128]
        w = np.asarray(w, np.float32)
        a = w.transpose(2, 0, 1).reshape(kc, 128, 2, 12, 128)
        return np.ascontiguousarray(a.transpose(1, 0, 2, 3, 4)).astype(bft)

    WihT0 = wT(inputs["w_ih0"], 4)
    WhhT0 = wT(inputs["w_hh0"], 4)
    WihT1 = wT(inputs["w_ih1"], 8)
    WhhT1 = wT(inputs["w_hh1"], 4)

    # biasT[slot, layer, dir, g]
    biasT = np.zeros((16, 2, 2, 128), np.float32)
    for l, (bi, bh) in enumerate([
            (np.asarray(inputs["b_ih0"], np.float32),
             np.asarray(inputs["b_hh0"], np.float32)),
            (np.asarray(inputs["b_ih1"], np.float32),
             np.asarray(inputs["b_hh1"], np.float32))]):
        for d in range(2):
            rz = (bi[d, :1024] + bh[d, :1024]).reshape(8, 128)
            biasT[0:8, l, d, :] = rz
            biasT[8:12, l, d, :] = bh[d, 1024:].reshape(4, 128)
            biasT[12:16, l, d, :] = bi[d, 1024:].reshape(4, 128)
    biasT = biasT.astype(bft)

    sel = np.zeros((16, 16, W, BPC), np.float32)
    for s in range(16):
        sel[s, s] = 1.0
    sel = sel.astype(bft)

    fcw_pad = np.zeros((VPAD, 2 * H), np.float32)
    fcw_pad[:V] = fc_w
    fcb_pad = np.full((VPAD,), PADB, np.float32)
    fcb_pad[:V] = fc_b * (SW * SX)

    ones8 = np.zeros((1, 2, 128), np.float32)
    ones8[0, 0, :] = 1.0
    ones8 = ones8.astype(f8t)

    emb = emb_t[tgt]                      # [B, S, E]
    ctx4 = ctx.reshape(L, 2, B, H)        # [l, d, b, h]

    in_maps = []
    for c in range(NC_):
        bs = slice(BPC * c, BPC * (c + 1))
        emb_rows = emb[bs].transpose(1, 0, 2).reshape(R, E)   # row = 4t+b
        embT = np.ascontiguousarray(
            emb_rows.T.reshape(4, 128, R).transpose(1, 0, 2)).astype(bft)
        cc = ctx4[:, :, bs, :]                                # [l, d, 4, h]
        h0a = cc.transpose(3, 0, 1, 2).reshape(4, 128, L, 2, BPC)
        h0T = np.ascontiguousarray(
            h0a.transpose(1, 2, 3, 0, 4)).astype(bft)
        shard = fcw_pad[VS * c:VS * (c + 1)] * SW             # [VS, 1024]
        fcw8 = np.ascontiguousarray(
            shard.T.reshape(8, 128, VS).transpose(1, 0, 2)).astype(f8t)
        fcb8 = np.zeros((1, 4, 2, 1024), np.float32)
        fcb8[0, :, 0, :] = fcb_pad[VS * c:VS * (c + 1)].reshape(4, 1024)
        fcb8 = fcb8.astype(f8t)
        in_maps.append({
            "embT": embT, "h0T": h0T,
            "WihT0": WihT0, "WhhT0": WhhT0,
            "WihT1": WihT1, "WhhT1": WhhT1,
            "biasT": biasT, "sel16": sel,
            "fcw8": fcw8, "fcb8": fcb8, "ones8": ones8,
        })
    return in_maps


def _unshard(results):
    Lfull = np.concatenate(
        [results[c]["out"].astype(np.float32) for c in range(NC_)], axis=1)
    Lfull = Lfull[:, :V]                  # [4096, 32000]
    b = np.arange(B)[:, None]
    s = np.arange(S)[None, :]
    rows = (b // BPC) * R + BPC * s + (b % BPC)
    return Lfull[rows]                    # [B, S, V]


def kernel(**inputs):
    from concourse.bass_utils import run_bass_kernel_spmd
    nc = _get_nc()
    in_maps = _prep_inputs(inputs)
    res = run_bass_kernel_spmd(nc, in_maps, core_ids=list(range(NC_)))
    return _unshard(res.results)


# revision 21
# speedup vs baseline: 1.1984x; 1.0203x over previous
"""Trainium2 Bass kernel for nn_PlainDecoder (2-layer 2-dir GRU decoder +
vocab projection + log_softmax).

Sharding: data-parallel over batch (4 per core) for the scan; vocab-parallel
(4096-wide shard of padded 32768) for the logits.

Scan design (transposed orientation): all gate matmuls output
[128 gate-partitions, 4 batch] so the PE bill (prop. to output FREE size) is
tiny and h' is produced directly in lhsT (hidden-major) layout -- no PE
transposes.  Per (layer, dir) a windowed PSUM tile P holds, per step, 16
slots of 128 gates: [0:8]=r|z (preloaded with gi+bias), [8:12]=n-gh
(preloaded with b_hh_n), [12:16]=gi_n (+b_ih_n).  A bias matmul opens each
window bank (start=True), the windowed gi GEMM and the per-step Whh matmuls
accumulate on top (start=False).  Both layers run interleaved (layer 1 lags
LAG steps).  Everything bf16 into the PE, f32 in PSUM.

Logits: x2 (= layer-1 hist, bf16) is scaled to fp8 and AllGather'd; fc_w is
fp8.  Matmuls run in DoubleRow perf mode (K=256/instr, 0.5 cyc/row).  Per
(128-row block, 1024-vocab chunk): exp(l/256) with accumulated row sums and
a bf16 copy of l (frees PSUM fast); one AllReduce of partial sums per block;
out = l - ln(S) written bf16 (host converts to f32).
"""

import os
import sys
from contextlib import ExitStack

for _p in ("/opt/trn_rl_repo", "/root/.axon_site/_ro/trn_rl_repo"):
    if os.path.isdir(_p) and _p not in sys.path:
        sys.path.insert(0, _p)

import numpy as np  # noqa: E402
import ml_dtypes  # noqa: E402

V, E, H, L, B, S = 32000, 512, 512, 2, 32, 128
NC_ = 8                      # cores
BPC = B // NC_               # batches per core = 4
R = BPC * S                  # rows per core = 512 (s-major: row = 4*t + b)
VPAD = 32768
VS = VPAD // NC_             # vocab shard per core = 4096
W = 8                        # scan PSUM window (steps)
LAG = 12                     # layer-1 lag (steps)
NW = S // W                  # 16 windows
SW = 64.0                    # fc_w fp8 scale
SX = 4.0                     # x2 fp8 scale
SREC = 1.0 / (SW * SX)       # logits descale
PADB = -240.0                # pad-vocab scaled bias (e4m3 max finite)
NROW = NC_ * R               # 4096 global rows
NBLK = NROW // 128           # 32 row blocks

_BUILT = {}


def _build_nc(n_cores=NC_, sim=False):
    """Build the Bass program (same NEFF for all cores; per-core data
    differs).  sim=True replaces collectives with local DMAs so TimelineSim
    can run."""
    import concourse.bass as bass  # noqa: F401
    import concourse.mybir as mybir
    import concourse.tile as tile
    from concourse import bacc

    dt = mybir.dt
    f32 = dt.float32
    bf = dt.bfloat16
    f8 = dt.float8e4
    AF = mybir.ActivationFunctionType
    OP = mybir.AluOpType
    PM = mybir.MatmulPerfMode

    nc = bacc.Bacc("TRN2", target_bir_lowering=False, debug=False,
                   num_devices=n_cores)

    # ---------------- DRAM I/O ----------------
    embT = nc.dram_tensor("embT", [128, 4, R], bf, kind="ExternalInput")
    h0T = nc.dram_tensor("h0T", [128, 2, 2, 4, BPC], bf, kind="ExternalInput")
    WihT0 = nc.dram_tensor("WihT0", [128, 4, 2, 12, 128], bf,
                           kind="ExternalInput")
    WhhT0 = nc.dram_tensor("WhhT0", [128, 4, 2, 12, 128], bf,
                           kind="ExternalInput")
    WihT1 = nc.dram_tensor("WihT1", [128, 8, 2, 12, 128], bf,
                           kind="ExternalInput")
    WhhT1 = nc.dram_tensor("WhhT1", [128, 4, 2, 12, 128], bf,
                           kind="ExternalInput")
    biasT = nc.dram_tensor("biasT", [16, 2, 2, 128], bf, kind="ExternalInput")
    sel16 = nc.dram_tensor("sel16", [16, 16, W, BPC], bf, kind="ExternalInput")
    fcw8 = nc.dram_tensor("fcw8", [128, 8, VS], f8, kind="ExternalInput")
    fcb8 = nc.dram_tensor("fcb8", [1, 4, 2, 1024], f8, kind="ExternalInput")
    ones8 = nc.dram_tensor("ones8", [1, 2, 128], f8, kind="ExternalInput")

    out_d = nc.dram_tensor("out", [NROW, VS], bf, kind="ExternalOutput")

    # internal DRAM for collectives
    HR = R // 2
    agx_in = [nc.dram_tensor(f"agx_in{h}", [128, 8, HR], f8, kind="Internal")
              for h in range(2)]
    agx_out = [nc.dram_tensor(f"agx_out{h}", [n_cores * 128, 8, HR], f8,
                              kind="Internal", addr_space="Shared")
               for h in range(2)]
    NGRP = NBLK // 4          # 8 sum-collective groups of 4 row blocks
    ags_in = [nc.dram_tensor(f"ags_in{g}", [128, 4], f32, kind="Internal")
              for g in range(NGRP)]
    ags_out = [nc.dram_tensor(f"ags_out{g}", [128, 4], f32,
                              kind="Internal", addr_space="Shared")
               for g in range(NGRP)]
    rg = [list(range(n_cores))]

    with tile.TileContext(nc) as tc, ExitStack() as top:
        # logits weights: loaded up front so the DMAs ride under the scan
        fwpool = top.enter_context(tc.tile_pool(name="fw", bufs=1))
        fw = fwpool.tile([128, 8, VS], f8, tag="fw", name="fw")
        nc.sync.dma_start(fw[:], fcw8[:])
        fb = fwpool.tile([1, 4, 2, 1024], f8, tag="fb", name="fb")
        nc.sync.dma_start(fb[:], fcb8[:])
        on8 = fwpool.tile([1, 2, 128], f8, tag="on8", name="on8")
        nc.sync.dma_start(on8[:], ones8[:])

        # ---------------- scan phase ----------------
        with ExitStack() as scan_stack:
            wpool = scan_stack.enter_context(tc.tile_pool(name="wts", bufs=1))
            hpool = scan_stack.enter_context(tc.tile_pool(name="hist", bufs=1))
            cpool = scan_stack.enter_context(tc.tile_pool(name="chain", bufs=3))
            p0pool = scan_stack.enter_context(
                tc.tile_pool(name="P0", bufs=2, space="PSUM"))
            p1pool = scan_stack.enter_context(
                tc.tile_pool(name="P1", bufs=2, space="PSUM"))

            embT_sb = wpool.tile([128, 4, R], bf, tag="embT", name="embT_sb")
            nc.sync.dma_start(embT_sb[:], embT[:])
            h0_sb = wpool.tile([128, 2, 2, 4, BPC], bf, tag="h0", name="h0_sb")
            nc.sync.dma_start(h0_sb[:], h0T[:])
            wih0 = wpool.tile([128, 4, 2, 12, 128], bf, tag="wih0",
                              name="wih0")
            nc.sync.dma_start(wih0[:], WihT0[:])
            whh0 = wpool.tile([128, 4, 2, 12, 128], bf, tag="whh0",
                              name="whh0")
            nc.sync.dma_start(whh0[:], WhhT0[:])
            wih1 = wpool.tile([128, 8, 2, 12, 128], bf, tag="wih1",
                              name="wih1")
            nc.sync.dma_start(wih1[:], WihT1[:])
            whh1 = wpool.tile([128, 4, 2, 12, 128], bf, tag="whh1",
                              name="whh1")
            nc.sync.dma_start(whh1[:], WhhT1[:])
            bias_sb = wpool.tile([16, 2, 2, 128], bf, tag="bias",
                                 name="bias_sb")
            nc.sync.dma_start(bias_sb[:], biasT[:])
            sel_sb = wpool.tile([16, 16, W, BPC], bf, tag="sel", name="sel_sb")
            nc.sync.dma_start(sel_sb[:], sel16[:])

            # hist layout: [128 h-part, dir, kchunk, row(=4t+b)]
            hist = [hpool.tile([128, 2, 4, R], bf, tag=f"hist{l}",
                               name=f"hist{l}") for l in range(2)]
            ppool = [p0pool, p1pool]
            wih = [wih0, wih1]
            whh = [whh0, whh1]
            kc_ih = [4, 8]
            pwin = [{}, {}]       # (layer, window) -> PSUM tile
            gwin = [{}, {}]       # (layer, window) -> SBUF gi_n tile

            def gi_window(l, w):
                """Bias opener + gi GEMM for window w of layer l.
                P layout: [128, dir, slot16, W, BPC]."""
                P = ppool[l].tile([128, 2, 16, W, BPC], f32, tag=f"P{l}",
                                  name=f"P{l}w{w}")
                pwin[l][w] = P
                rows = slice(BPC * W * w, BPC * W * (w + 1))
                for d in range(2):
                    nc.tensor.matmul(P[:, d], bias_sb[:, l, d, :], sel_sb[:],
                                     start=True, stop=False,
                                     skip_group_check=True)
                for d in range(2):
                    for sl in range(12):
                        slot = sl if sl < 8 else sl + 4
                        for k in range(kc_ih[l]):
                            if l == 0:
                                rhs = embT_sb[:, k, rows]
                            else:
                                rhs = hist[0][:, k // 4, k % 4, rows]
                            nc.tensor.matmul(
                                P[:, d, slot], wih[l][:, k, d, sl, :], rhs,
                                start=False,
                                stop=(slot >= 12 and k == kc_ih[l] - 1),
                                skip_group_check=True)
                # stage gi_n into SBUF so the in-loop add is SBUF-only
                gin = cpool.tile([128, 2, 4, W, BPC], f32, tag=f"gin{l}",
                                 bufs=2, name=f"gin{l}")
                nc.vector.tensor_copy(gin[:], P[:, :, 12:16, :, :])
                gwin[l][w] = gin

            def scan_step(l, t):
                """Whh matmuls + GRU cell chain for step t of layer l."""
                P = pwin[l][t // W]
                gin = gwin[l][t // W]
                tw = t % W
                if t == 0:
                    hp = h0_sb[:, l]                      # [128, 2, 4, BPC]
                else:
                    hp = hist[l][:, :, :, BPC * (t - 1):BPC * t]
                # k-major, r|z slices first: k0/k1 matmuls wait only on the
                # first half of the h' tail, and the sigmoid (head of the
                # serial chain) doesn't wait on the n-gh matmuls
                for j0, j1 in ((0, 8), (8, 12)):
                    for k in range(4):
                        for d in range(2):
                            for j in range(j0, j1):
                                nc.tensor.matmul(
                                    P[:, d, j, tw, :], whh[l][:, k, d, j, :],
                                    hp[:, d, k, :], start=False,
                                    stop=(k == 3), skip_group_check=True)
                # GRU cell chain (both dirs in one op each):
                #   h' = n*(1-z) + z*hprev ; q=z*hprev and u=1-z leave the
                #   serial path right after the sigmoid
                rzs = cpool.tile([128, 2, 8, BPC], f32, tag=f"rzs{l}",
                                 name=f"rzs{l}")
                nc.scalar.activation(rzs[:], P[:, :, 0:8, tw, :], AF.Sigmoid)
                n1 = cpool.tile([128, 2, 4, BPC], f32, tag=f"n1{l}",
                                name=f"n1{l}")
                nc.vector.tensor_mul(n1[:], P[:, :, 8:12, tw, :],
                                     rzs[:, :, 0:4, :])
                nc.vector.tensor_add(n1[:], n1[:], gin[:, :, :, tw, :])
                q = cpool.tile([128, 2, 4, BPC], f32, tag=f"q{l}",
                               name=f"q{l}")
                nc.vector.tensor_mul(q[:], rzs[:, :, 4:8, :], hp[:])
                u = cpool.tile([128, 2, 4, BPC], f32, tag=f"u{l}",
                               name=f"u{l}")
                nc.vector.tensor_scalar(u[:], rzs[:, :, 4:8, :], -1.0, 1.0,
                                        OP.mult, OP.add)
                nt = cpool.tile([128, 2, 4, BPC], f32, tag=f"nt{l}",
                                name=f"nt{l}")
                nc.scalar.activation(nt[:], n1[:], AF.Tanh)
                # tail on Pool (no ack latency, SBUF-only operands), split per
                # k-half so next-step k0/k1 matmuls start before k2/k3 finish
                d1 = cpool.tile([128, 2, 4, BPC], f32, tag=f"d1{l}",
                                name=f"d1{l}")
                for ks in (slice(0, 2), slice(2, 4)):
                    nc.gpsimd.tensor_mul(d1[:, :, ks], nt[:, :, ks],
                                         u[:, :, ks])
                    nc.gpsimd.tensor_add(
                        hist[l][:, :, ks, BPC * t:BPC * (t + 1)],
                        d1[:, :, ks], q[:, :, ks])

            def ship_x2(h):
                rows = slice(HR * h, HR * (h + 1))
                x8 = cpool.tile([128, 2, 4, HR], f8, tag="x8", bufs=2,
                                name="x8")
                nc.vector.tensor_scalar_mul(x8[:], hist[1][:, :, :, rows], SX)
                nc.sync.dma_start(agx_in[h][:],
                                  x8[:].rearrange("p d k r -> p (d k) r"))
                if sim:
                    nc.sync.dma_start(agx_out[h][0:128], agx_in[h][:])
                else:
                    nc.gpsimd.collective_compute(
                        "AllGather", OP.bypass, replica_groups=rg,
                        ins=[agx_in[h][:].opt()], outs=[agx_out[h][:].opt()])

            gi_window(0, 0)
            for it in range(S + LAG):
                if it == S // 2 + LAG + 1:
                    ship_x2(0)        # L1 rows 0..255 done; gather them now
                if it % W == 5 and (it + 3) // W < NW:
                    gi_window(0, (it + 3) // W)
                if it % W == 1 and it >= 9 and (it - 9) // W < NW:
                    gi_window(1, (it - 9) // W)
                if it < S:
                    scan_step(0, it)
                t1 = it - LAG
                if 0 <= t1 < S:
                    scan_step(1, t1)

            ship_x2(1)

        # ---------------- logits + log_softmax ----------------
        with (
            tc.tile_pool(name="lt", bufs=3) as ltpool,
            tc.tile_pool(name="lps", bufs=4, space="PSUM") as lpspool,
            tc.tile_pool(name="lsc", bufs=2) as lscpool,
        ):
            x2g = ltpool.tile([128, 8, 8, R], f8, tag="x2g", bufs=1,
                              name="x2g")
            # per-source-core, per-half chunks: contiguous DMAs; the first
            # half is gathered mid-scan, so early row blocks start sooner
            for c in range(8):
                for h in range(2):
                    nc.sync.dma_start(
                        x2g[:, :, c, HR * h:HR * (h + 1)],
                        agx_out[h][128 * c:128 * (c + 1), :, :])

            def block_mm(rb, lb, sgrp, rb4):
                """Matmuls + exp/sums + bf16-l copy for one 128-row block.
                All 40 matmuls are emitted back-to-back (keeps the PE
                p-state ramped); exp/copies follow."""
                csrc, r0 = rb // BPC, (rb % BPC) * 128
                srb = lscpool.tile([128, 4], f32, tag="srb", bufs=3,
                                   name="srb")
                Ps = []
                for vq in range(4):
                    P = lpspool.tile([128, 1024], f32, tag="lp", name="lp")
                    Ps.append(P)
                    # matmuls per 512-wide half: a PSUM matmul target must
                    # stay within one 2KB bank
                    for hh in range(2):
                        v0 = 1024 * vq + 512 * hh
                        nc.tensor.matmul(P[:, 512 * hh:512 * (hh + 1)],
                                         on8[:], fb[0:1, vq, :,
                                                    512 * hh:512 * (hh + 1)],
                                         start=True, stop=False,
                                         perf_mode=PM.DoubleRow,
                                         skip_group_check=True)
                        for c2 in range(4):
                            nc.tensor.matmul(
                                P[:, 512 * hh:512 * (hh + 1)],
                                x2g[:, 2 * c2:2 * c2 + 2, csrc, r0:r0 + 128],
                                fw[:, 2 * c2:2 * c2 + 2, v0:v0 + 512],
                                start=False, stop=(c2 == 3),
                                perf_mode=PM.DoubleRow, skip_group_check=True)
                for vq in range(4):
                    eb = lscpool.tile([128, 1024], bf, tag="eb", bufs=3,
                                      name="eb")
                    nc.scalar.activation(eb[:], Ps[vq][:], AF.Exp, scale=SREC,
                                         accum_out=srb[:, vq:vq + 1])
                    # bf16 copy of l frees the PSUM bank quickly
                    nc.vector.tensor_scalar_mul(lb[:, vq, :], Ps[vq][:], SREC)
                nc.vector.tensor_reduce(sgrp[:, rb4:rb4 + 1], srb[:],
                                        axis=mybir.AxisListType.X, op=OP.add)

            for g in range(NGRP):
                lbs = []
                obs = []
                sgrp = lscpool.tile([128, 4], f32, tag="sgrp", bufs=3,
                                    name="sgrp")
                for rb4 in range(4):
                    lb = ltpool.tile([128, 4, 1024], bf, tag="lb", bufs=6,
                                     name="lb")
                    block_mm(4 * g + rb4, lb, sgrp, rb4)
                    lbs.append(lb)
                nc.sync.dma_start(ags_in[g][:], sgrp[:])
                if sim:
                    nc.sync.dma_start(ags_out[g][:], ags_in[g][:])
                else:
                    nc.gpsimd.collective_compute(
                        "AllReduce", OP.add, replica_groups=rg,
                        ins=[ags_in[g][:].opt()],
                        outs=[ags_out[g][:].opt()])
                sg = lscpool.tile([128, 4], f32, tag="sg", name="sg")
                nc.sync.dma_start(sg[:], ags_out[g][:])
                lnS = lscpool.tile([128, 4], f32, tag="lnS", name="lnS")
                nc.scalar.activation(lnS[:], sg[:], AF.Ln)
                negS = lscpool.tile([128, 4], f32, tag="negS", name="negS")
                nc.vector.tensor_scalar_mul(negS[:], lnS[:], -1.0)
                for rb4 in range(4):
                    ob = ltpool.tile([128, VS], bf, tag="ob", bufs=4,
                                     name="ob")
                    for vq in range(4):
                        # balance the subtract across Pool / DVE / ACT
                        if vq < 2:
                            nc.gpsimd.tensor_scalar_sub(
                                ob[:, 1024 * vq:1024 * (vq + 1)],
                                lbs[rb4][:, vq, :], lnS[:, rb4:rb4 + 1])
                        elif vq == 2:
                            nc.vector.tensor_scalar_sub(
                                ob[:, 1024 * vq:1024 * (vq + 1)],
                                lbs[rb4][:, vq, :], lnS[:, rb4:rb4 + 1])
                        else:
                            nc.scalar.activation(
                                ob[:, 1024 * vq:1024 * (vq + 1)],
                                lbs[rb4][:, vq, :], AF.Identity,
                                bias=negS[:, rb4:rb4 + 1])
                    obs.append(ob)
                for rb4 in range(4):
                    rb = 4 * g + rb4
                    nc.sync.dma_start(out_d[128 * rb:128 * (rb + 1), :],
                                      obs[rb4][:])

    nc.compile()
    return nc


def _get_nc():
    if "nc" not in _BUILT:
        _BUILT["nc"] = _build_nc()
    return _BUILT["nc"]


def _prep_inputs(inputs):
    """Host-side shard + relayout. Returns in_maps for 8 cores."""
    bft = ml_dtypes.bfloat16
    f8t = ml_dtypes.float8_e4m3

    tgt = np.asarray(inputs["target"])
    ctx = np.asarray(inputs["context"], np.float32)
    emb_t = np.asarray(inputs["embed_table"], np.float32)
    fc_w = np.asarray(inputs["fc_w"], np.float32)
    fc_b = np.asarray(inputs["fc_b"], np.float32)

    def wT(w, kc):     # [2, 1536, IN] -> [128, kc, 2, 12, 128]
        w = np.asarray(w, np.float32)
        a = w.transpose(2, 0, 1).reshape(kc, 128, 2, 12, 128)
        return np.ascontiguousarray(a.transpose(1, 0, 2, 3, 4)).astype(bft)

    WihT0 = wT(inputs["w_ih0"], 4)
    WhhT0 = wT(inputs["w_hh0"], 4)
    WihT1 = wT(inputs["w_ih1"], 8)
    WhhT1 = wT(inputs["w_hh1"], 4)

    # biasT[slot, layer, dir, g]
    biasT = np.zeros((16, 2, 2, 128), np.float32)
    for l, (bi, bh) in enumerate([
            (np.asarray(inputs["b_ih0"], np.float32),
             np.asarray(inputs["b_hh0"], np.float32)),
            (np.asarray(inputs["b_ih1"], np.float32),
             np.asarray(inputs["b_hh1"], np.float32))]):
        for d in range(2):
            rz = (bi[d, :1024] + bh[d, :1024]).reshape(8, 128)
            biasT[0:8, l, d, :] = rz
            biasT[8:12, l, d, :] = bh[d, 1024:].reshape(4, 128)
            biasT[12:16, l, d, :] = bi[d, 1024:].reshape(4, 128)
    biasT = biasT.astype(bft)

    sel = np.zeros((16, 16, W, BPC), np.float32)
    for s in range(16):
        sel[s, s] = 1.0
    sel = sel.astype(bft)

    fcw_pad = np.zeros((VPAD, 2 * H), np.float32)
    fcw_pad[:V] = fc_w
    fcb_pad = np.full((VPAD,), PADB, np.float32)
    fcb_pad[:V] = fc_b * (SW * SX)

    ones8 = np.zeros((1, 2, 128), np.float32)
    ones8[0, 0, :] = 1.0
    ones8 = ones8.astype(f8t)

    emb = emb_t[tgt]                      # [B, S, E]
    ctx4 = ctx.reshape(L, 2, B, H)        # [l, d, b, h]

    in_maps = []
    for c in range(NC_):
        bs = slice(BPC * c, BPC * (c + 1))
        emb_rows = emb[bs].transpose(1, 0, 2).reshape(R, E)   # row = 4t+b
        embT = np.ascontiguousarray(
            emb_rows.T.reshape(4, 128, R).transpose(1, 0, 2)).astype(bft)
        cc = ctx4[:, :, bs, :]                                # [l, d, 4, h]
        h0a = cc.transpose(3, 0, 1, 2).reshape(4, 128, L, 2, BPC)
        h0T = np.ascontiguousarray(
            h0a.transpose(1, 2, 3, 0, 4)).astype(bft)
        shard = fcw_pad[VS * c:VS * (c + 1)] * SW             # [VS, 1024]
        fcw8 = np.ascontiguousarray(
            shard.T.reshape(8, 128, VS).transpose(1, 0, 2)).astype(f8t)
        fcb8 = np.zeros((1, 4, 2, 1024), np.float32)
        fcb8[0, :, 0, :] = fcb_pad[VS * c:VS * (c + 1)].reshape(4, 1024)
        fcb8 = fcb8.astype(f8t)
        in_maps.append({
            "embT": embT, "h0T": h0T,
            "WihT0": WihT0, "WhhT0": WhhT0,
            "WihT1": WihT1, "WhhT1": WhhT1,
            "biasT": biasT, "sel16": sel,
            "fcw8": fcw8, "fcb8": fcb8, "ones8": ones8,
        })
    return in_maps


def _unshard(results):
    Lfull = np.concatenate(
        [results[c]["out"].astype(np.float32) for c in range(NC_)], axis=1)
    Lfull = Lfull[:, :V]                  # [4096, 32000]
    b = np.arange(B)[:, None]
    s = np.arange(S)[None, :]
    rows = (b // BPC) * R + BPC * s + (b % BPC)
    return Lfull[rows]                    # [B, S, V]


def kernel(**inputs):
    from concourse.bass_utils import run_bass_kernel_spmd
    nc = _get_nc()
    in_maps = _prep_inputs(inputs)
    res = run_bass_kernel_spmd(nc, in_maps, core_ids=list(range(NC_)))
    return _unshard(res.results)


# revision 29
# speedup vs baseline: 1.2312x; 1.0274x over previous
"""Trainium2 Bass kernel for nn_PlainDecoder (2-layer 2-dir GRU decoder +
vocab projection + log_softmax).

Sharding: data-parallel over batch (4 per core) for the scan; vocab-parallel
(4096-wide shard of padded 32768) for the logits.

Scan design (transposed orientation): all gate matmuls output
[128 gate-partitions, 4 batch] so the PE bill (prop. to output FREE size) is
tiny and h' is produced directly in lhsT (hidden-major) layout -- no PE
transposes.  Per (layer, dir) a windowed PSUM tile P holds, per step, 16
slots of 128 gates: [0:8]=r|z (preloaded with gi+bias), [8:12]=n-gh
(preloaded with b_hh_n), [12:16]=gi_n (+b_ih_n).  A bias matmul opens each
window bank (start=True), the windowed gi GEMM and the per-step Whh matmuls
accumulate on top (start=False).  Both layers run interleaved (layer 1 lags
LAG steps).  Everything bf16 into the PE, f32 in PSUM.

Logits: x2 (= layer-1 hist, bf16) is scaled to fp8 and AllGather'd; fc_w is
fp8.  Matmuls run in DoubleRow perf mode (K=256/instr, 0.5 cyc/row).  Per
(128-row block, 1024-vocab chunk): exp(l/256) with accumulated row sums and
a bf16 copy of l (frees PSUM fast); one AllReduce of partial sums per block;
out = l - ln(S) written bf16 (host converts to f32).
"""

import os
import sys
from contextlib import ExitStack

for _p in ("/opt/trn_rl_repo", "/root/.axon_site/_ro/trn_rl_repo"):
    if os.path.isdir(_p) and _p not in sys.path:
        sys.path.insert(0, _p)

import numpy as np  # noqa: E402
import ml_dtypes  # noqa: E402

V, E, H, L, B, S = 32000, 512, 512, 2, 32, 128
NC_ = 8                      # cores
BPC = B // NC_               # batches per core = 4
R = BPC * S                  # rows per core = 512 (s-major: row = 4*t + b)
VPAD = 32768
VS = VPAD // NC_             # vocab shard per core = 4096
W = 8                        # scan PSUM window (steps)
LAG = 12                     # layer-1 lag (steps)
NW = S // W                  # 16 windows
SW = 64.0                    # fc_w fp8 scale
SX = 4.0                     # x2 fp8 scale
SREC = 1.0 / (SW * SX)       # logits descale
PADB = -240.0                # pad-vocab scaled bias (e4m3 max finite)
NROW = NC_ * R               # 4096 global rows
NBLK = NROW // 128           # 32 row blocks

_BUILT = {}


def _build_nc(n_cores=NC_, sim=False):
    """Build the Bass program (same NEFF for all cores; per-core data
    differs).  sim=True replaces collectives with local DMAs so TimelineSim
    can run."""
    import concourse.bass as bass  # noqa: F401
    import concourse.mybir as mybir
    import concourse.tile as tile
    from concourse import bacc

    dt = mybir.dt
    f32 = dt.float32
    bf = dt.bfloat16
    f8 = dt.float8e4
    AF = mybir.ActivationFunctionType
    OP = mybir.AluOpType
    PM = mybir.MatmulPerfMode

    nc = bacc.Bacc("TRN2", target_bir_lowering=False, debug=False,
                   num_devices=n_cores)

    # ---------------- DRAM I/O ----------------
    embT = nc.dram_tensor("embT", [128, 4, R], bf, kind="ExternalInput")
    h0T = nc.dram_tensor("h0T", [128, 2, 2, 4, BPC], bf, kind="ExternalInput")
    WihT0 = nc.dram_tensor("WihT0", [128, 4, 2, 12, 128], bf,
                           kind="ExternalInput")
    WhhT0 = nc.dram_tensor("WhhT0", [128, 4, 2, 12, 128], bf,
                           kind="ExternalInput")
    WihT1 = nc.dram_tensor("WihT1", [128, 8, 2, 12, 128], bf,
                           kind="ExternalInput")
    WhhT1 = nc.dram_tensor("WhhT1", [128, 4, 2, 12, 128], bf,
                           kind="ExternalInput")
    biasT = nc.dram_tensor("biasT", [16, 2, 2, 128], bf, kind="ExternalInput")
    sel16 = nc.dram_tensor("sel16", [16, 16, W, BPC], bf, kind="ExternalInput")
    fcw8 = nc.dram_tensor("fcw8", [128, 8, VS], f8, kind="ExternalInput")
    fcb8 = nc.dram_tensor("fcb8", [1, 4, 2, 1024], f8, kind="ExternalInput")
    ones8 = nc.dram_tensor("ones8", [1, 2, 128], f8, kind="ExternalInput")

    out_d = nc.dram_tensor("out", [NROW, VS], bf, kind="ExternalOutput")

    # internal DRAM for collectives
    HR = R // 2
    agx_in = [nc.dram_tensor(f"agx_in{h}", [128, 8, HR], f8, kind="Internal")
              for h in range(2)]
    agx_out = [nc.dram_tensor(f"agx_out{h}", [n_cores * 128, 8, HR], f8,
                              kind="Internal", addr_space="Shared")
               for h in range(2)]
    NGRP = NBLK // 4          # 8 sum-collective groups of 4 row blocks
    ags_in = [nc.dram_tensor(f"ags_in{g}", [128, 4], f32, kind="Internal")
              for g in range(NGRP)]
    ags_out = [nc.dram_tensor(f"ags_out{g}", [128, 4], f32,
                              kind="Internal", addr_space="Shared")
               for g in range(NGRP)]
    rg = [list(range(n_cores))]

    with tile.TileContext(nc) as tc, ExitStack() as top:
        # logits weights pool (DMAs emitted after the scan weights so they
        # ride under the scan instead of delaying its start)
        fwpool = top.enter_context(tc.tile_pool(name="fw", bufs=1))
        fw = fwpool.tile([128, 8, VS], f8, tag="fw", name="fw")
        fb = fwpool.tile([1, 4, 2, 1024], f8, tag="fb", name="fb")
        on8 = fwpool.tile([1, 2, 128], f8, tag="on8", name="on8")

        # ---------------- scan phase ----------------
        with ExitStack() as scan_stack:
            wpool = scan_stack.enter_context(tc.tile_pool(name="wts", bufs=1))
            hpool = scan_stack.enter_context(tc.tile_pool(name="hist", bufs=1))
            cpool = scan_stack.enter_context(tc.tile_pool(name="chain", bufs=3))
            p0pool = scan_stack.enter_context(
                tc.tile_pool(name="P0", bufs=2, space="PSUM"))
            p1pool = scan_stack.enter_context(
                tc.tile_pool(name="P1", bufs=2, space="PSUM"))

            embT_sb = wpool.tile([128, 4, R], bf, tag="embT", name="embT_sb")
            nc.sync.dma_start(embT_sb[:], embT[:])
            h0_sb = wpool.tile([128, 2, 2, 4, BPC], bf, tag="h0", name="h0_sb")
            nc.sync.dma_start(h0_sb[:], h0T[:])
            wih0 = wpool.tile([128, 4, 2, 12, 128], bf, tag="wih0",
                              name="wih0")
            nc.sync.dma_start(wih0[:], WihT0[:])
            whh0 = wpool.tile([128, 4, 2, 12, 128], bf, tag="whh0",
                              name="whh0")
            nc.sync.dma_start(whh0[:], WhhT0[:])
            wih1 = wpool.tile([128, 8, 2, 12, 128], bf, tag="wih1",
                              name="wih1")
            nc.sync.dma_start(wih1[:], WihT1[:])
            whh1 = wpool.tile([128, 4, 2, 12, 128], bf, tag="whh1",
                              name="whh1")
            nc.sync.dma_start(whh1[:], WhhT1[:])
            bias_sb = wpool.tile([16, 2, 2, 128], bf, tag="bias",
                                 name="bias_sb")
            nc.sync.dma_start(bias_sb[:], biasT[:])
            sel_sb = wpool.tile([16, 16, W, BPC], bf, tag="sel", name="sel_sb")
            nc.sync.dma_start(sel_sb[:], sel16[:])
            nc.sync.dma_start(fw[:], fcw8[:])
            nc.sync.dma_start(fb[:], fcb8[:])
            nc.sync.dma_start(on8[:], ones8[:])

            # hist layout: [128 h-part, dir, kchunk, row(=4t+b)]
            hist = [hpool.tile([128, 2, 4, R], bf, tag=f"hist{l}",
                               name=f"hist{l}") for l in range(2)]
            ppool = [p0pool, p1pool]
            wih = [wih0, wih1]
            whh = [whh0, whh1]
            kc_ih = [4, 8]
            pwin = [{}, {}]       # (layer, window) -> PSUM tile
            gwin = [{}, {}]       # (layer, window) -> SBUF gi_n tile

            def gi_window(l, w, dirs=(0, 1)):
                """Bias opener + gi GEMM for window w of layer l (emitted
                per dir so late windows can be sliced across iterations).
                P layout: [128, dir, slot16, W, BPC]."""
                if 0 in dirs:
                    P = ppool[l].tile([128, 2, 16, W, BPC], f32, tag=f"P{l}",
                                      name=f"P{l}w{w}")
                    pwin[l][w] = P
                    gin = cpool.tile([128, 2, 4, W, BPC], f32, tag=f"gin{l}",
                                     bufs=2, name=f"gin{l}")
                    gwin[l][w] = gin
                P = pwin[l][w]
                gin = gwin[l][w]
                rows = slice(BPC * W * w, BPC * W * (w + 1))
                for d in dirs:
                    nc.tensor.matmul(P[:, d], bias_sb[:, l, d, :], sel_sb[:],
                                     start=True, stop=False,
                                     skip_group_check=True)
                    for sl in range(12):
                        slot = sl if sl < 8 else sl + 4
                        for k in range(kc_ih[l]):
                            if l == 0:
                                rhs = embT_sb[:, k, rows]
                            else:
                                rhs = hist[0][:, k // 4, k % 4, rows]
                            nc.tensor.matmul(
                                P[:, d, slot], wih[l][:, k, d, sl, :], rhs,
                                start=False,
                                stop=(slot >= 12 and k == kc_ih[l] - 1),
                                skip_group_check=True)
                    # stage gi_n into SBUF so the in-loop add is SBUF-only
                    nc.vector.tensor_copy(gin[:, d], P[:, d, 12:16, :, :])

            def scan_step(l, t):
                """Whh matmuls + GRU cell chain for step t of layer l."""
                P = pwin[l][t // W]
                gin = gwin[l][t // W]
                tw = t % W
                if t == 0:
                    hp = h0_sb[:, l]                      # [128, 2, 4, BPC]
                else:
                    hp = hist[l][:, :, :, BPC * (t - 1):BPC * t]
                # k-major, r|z slices first: k0/k1 matmuls wait only on the
                # first half of the h' tail, and the sigmoid (head of the
                # serial chain) doesn't wait on the n-gh matmuls
                for j0, j1 in ((0, 8), (8, 12)):
                    for k in range(4):
                        for d in range(2):
                            for j in range(j0, j1):
                                nc.tensor.matmul(
                                    P[:, d, j, tw, :], whh[l][:, k, d, j, :],
                                    hp[:, d, k, :], start=False,
                                    stop=(k == 3), skip_group_check=True)
                # GRU cell chain (both dirs in one op each):
                #   h' = n*(1-z) + z*hprev ; q=z*hprev and u=1-z leave the
                #   serial path right after the sigmoid
                rzs = cpool.tile([128, 2, 8, BPC], f32, tag=f"rzs{l}",
                                 name=f"rzs{l}")
                nc.scalar.activation(rzs[:], P[:, :, 0:8, tw, :], AF.Sigmoid)
                n1 = cpool.tile([128, 2, 4, BPC], f32, tag=f"n1{l}",
                                name=f"n1{l}")
                nc.vector.tensor_mul(n1[:], P[:, :, 8:12, tw, :],
                                     rzs[:, :, 0:4, :])
                nc.vector.tensor_add(n1[:], n1[:], gin[:, :, :, tw, :])
                q = cpool.tile([128, 2, 4, BPC], f32, tag=f"q{l}",
                               name=f"q{l}")
                nc.vector.tensor_mul(q[:], rzs[:, :, 4:8, :], hp[:])
                u = cpool.tile([128, 2, 4, BPC], f32, tag=f"u{l}",
                               name=f"u{l}")
                nc.vector.tensor_scalar(u[:], rzs[:, :, 4:8, :], -1.0, 1.0,
                                        OP.mult, OP.add)
                nt = cpool.tile([128, 2, 4, BPC], f32, tag=f"nt{l}",
                                name=f"nt{l}")
                nc.scalar.activation(nt[:], n1[:], AF.Tanh)
                # tail on Pool (no ack latency, SBUF-only operands), split per
                # k-half so next-step k0/k1 matmuls start before k2/k3 finish
                d1 = cpool.tile([128, 2, 4, BPC], f32, tag=f"d1{l}",
                                name=f"d1{l}")
                for ks in (slice(0, 2), slice(2, 4)):
                    nc.gpsimd.tensor_mul(d1[:, :, ks], nt[:, :, ks],
                                         u[:, :, ks])
                    nc.gpsimd.tensor_add(
                        hist[l][:, :, ks, BPC * t:BPC * (t + 1)],
                        d1[:, :, ks], q[:, :, ks])

            def ship_x2(h):
                rows = slice(HR * h, HR * (h + 1))
                x8 = cpool.tile([128, 2, 4, HR], f8, tag="x8", bufs=2,
                                name="x8")
                nc.vector.tensor_scalar_mul(x8[:], hist[1][:, :, :, rows], SX)
                nc.sync.dma_start(agx_in[h][:],
                                  x8[:].rearrange("p d k r -> p (d k) r"))
                if sim:
                    nc.sync.dma_start(agx_out[h][0:128], agx_in[h][:])
                else:
                    nc.gpsimd.collective_compute(
                        "AllGather", OP.bypass, replica_groups=rg,
                        ins=[agx_in[h][:].opt()], outs=[agx_out[h][:].opt()])

            gi_window(0, 0)
            for it in range(S + LAG):
                if it == S // 2 + LAG + 1:
                    ship_x2(0)        # L1 rows 0..255 done; gather them now
                if it % W == 5 and (it + 3) // W < NW:
                    w = (it + 3) // W
                    gi_window(0, w, dirs=(0, 1) if w < 14 else (0,))
                if it % W == 6 and (it + 2) // W >= 14 and (it + 2) // W < NW:
                    gi_window(0, (it + 2) // W, dirs=(1,))
                if it % W == 1 and it >= 9 and (it - 9) // W < NW:
                    w = (it - 9) // W
                    gi_window(1, w, dirs=(0, 1) if w < 12 else (0,))
                if it % W == 2 and it >= 10 and (it - 10) // W >= 12 and (it - 10) // W < NW:
                    gi_window(1, (it - 10) // W, dirs=(1,))
                if it < S:
                    scan_step(0, it)
                t1 = it - LAG
                if 0 <= t1 < S:
                    scan_step(1, t1)

            ship_x2(1)

        # ---------------- logits + log_softmax ----------------
        with (
            tc.tile_pool(name="lt", bufs=3) as ltpool,
            tc.tile_pool(name="lps", bufs=4, space="PSUM") as lpspool,
            tc.tile_pool(name="lsc", bufs=2) as lscpool,
        ):
            x2g = ltpool.tile([128, 8, 8, R], f8, tag="x2g", bufs=1,
                              name="x2g")
            # per-source-core, per-half chunks: contiguous DMAs; the first
            # half is gathered mid-scan, so early row blocks start sooner
            for c in range(8):
                for h in range(2):
                    nc.sync.dma_start(
                        x2g[:, :, c, HR * h:HR * (h + 1)],
                        agx_out[h][128 * c:128 * (c + 1), :, :])

            def block_mm(rb, lb, sgrp, rb4):
                """Matmuls + exp/sums + bf16-l copy for one 128-row block.
                All 40 matmuls are emitted back-to-back (keeps the PE
                p-state ramped); exp/copies follow."""
                csrc, r0 = rb // BPC, (rb % BPC) * 128
                srb = lscpool.tile([128, 4], f32, tag="srb", bufs=3,
                                   name="srb")
                def vq_mms(vq):
                    P = lpspool.tile([128, 1024], f32, tag="lp", name="lp")
                    # matmuls per 512-wide half: a PSUM matmul target must
                    # stay within one 2KB bank
                    for hh in range(2):
                        v0 = 1024 * vq + 512 * hh
                        nc.tensor.matmul(P[:, 512 * hh:512 * (hh + 1)],
                                         on8[:], fb[0:1, vq, :,
                                                    512 * hh:512 * (hh + 1)],
                                         start=True, stop=False,
                                         perf_mode=PM.DoubleRow,
                                         skip_group_check=True)
                        for c2 in range(4):
                            nc.tensor.matmul(
                                P[:, 512 * hh:512 * (hh + 1)],
                                x2g[:, 2 * c2:2 * c2 + 2, csrc, r0:r0 + 128],
                                fw[:, 2 * c2:2 * c2 + 2, v0:v0 + 512],
                                start=False, stop=(c2 == 3),
                                perf_mode=PM.DoubleRow, skip_group_check=True)
                    return P

                def vq_post(vq, P):
                    eb = lscpool.tile([128, 1024], bf, tag="eb", bufs=3,
                                      name="eb")
                    nc.scalar.activation(eb[:], P[:], AF.Exp, scale=SREC,
                                         accum_out=srb[:, vq:vq + 1])
                    # bf16 copy of l frees the PSUM bank quickly
                    nc.vector.tensor_scalar_mul(lb[:, vq, :], P[:], SREC)

                # software-pipelined: chunk vq's exp/copy emitted after chunk
                # vq+1's matmuls so the PE stream never waits on PSUM reuse
                Ps = [vq_mms(0)]
                for vq in range(1, 4):
                    Ps.append(vq_mms(vq))
                    vq_post(vq - 1, Ps[vq - 1])
                vq_post(3, Ps[3])
                nc.vector.tensor_reduce(sgrp[:, rb4:rb4 + 1], srb[:],
                                        axis=mybir.AxisListType.X, op=OP.add)

            for g in range(NGRP):
                lbs = []
                obs = []
                sgrp = lscpool.tile([128, 4], f32, tag="sgrp", bufs=3,
                                    name="sgrp")
                for rb4 in range(4):
                    lb = ltpool.tile([128, 4, 1024], bf, tag="lb", bufs=6,
                                     name="lb")
                    block_mm(4 * g + rb4, lb, sgrp, rb4)
                    lbs.append(lb)
                nc.sync.dma_start(ags_in[g][:], sgrp[:])
                if sim:
                    nc.sync.dma_start(ags_out[g][:], ags_in[g][:])
                else:
                    nc.gpsimd.collective_compute(
                        "AllReduce", OP.add, replica_groups=rg,
                        ins=[ags_in[g][:].opt()],
                        outs=[ags_out[g][:].opt()])
                sg = lscpool.tile([128, 4], f32, tag="sg", name="sg")
                nc.sync.dma_start(sg[:], ags_out[g][:])
                lnS = lscpool.tile([128, 4], f32, tag="lnS", name="lnS")
                nc.scalar.activation(lnS[:], sg[:], AF.Ln)
                negS = lscpool.tile([128, 4], f32, tag="negS", name="negS")
                nc.vector.tensor_scalar_mul(negS[:], lnS[:], -1.0)
                for rb4 in range(4):
                    ob = ltpool.tile([128, VS], bf, tag="ob", bufs=4,
                                     name="ob")
                    for vq in range(4):
                        # balance the subtract across Pool / DVE / ACT
                        if vq < 2:
                            nc.gpsimd.tensor_scalar_sub(
                                ob[:, 1024 * vq:1024 * (vq + 1)],
                                lbs[rb4][:, vq, :], lnS[:, rb4:rb4 + 1])
                        elif vq == 2:
                            nc.vector.tensor_scalar_sub(
                                ob[:, 1024 * vq:1024 * (vq + 1)],
                                lbs[rb4][:, vq, :], lnS[:, rb4:rb4 + 1])
                        else:
                            nc.scalar.activation(
                                ob[:, 1024 * vq:1024 * (vq + 1)],
                                lbs[rb4][:, vq, :], AF.Identity,
                                bias=negS[:, rb4:rb4 + 1])
                    obs.append(ob)
                for rb4 in range(4):
                    rb = 4 * g + rb4
                    nc.sync.dma_start(out_d[128 * rb:128 * (rb + 1), :],
                                      obs[rb4][:])

    nc.compile()
    return nc


def _get_nc():
    if "nc" not in _BUILT:
        _BUILT["nc"] = _build_nc()
    return _BUILT["nc"]


def _prep_inputs(inputs):
    """Host-side shard + relayout. Returns in_maps for 8 cores."""
    bft = ml_dtypes.bfloat16
    f8t = ml_dtypes.float8_e4m3

    tgt = np.asarray(inputs["target"])
    ctx = np.asarray(inputs["context"], np.float32)
    emb_t = np.asarray(inputs["embed_table"], np.float32)
    fc_w = np.asarray(inputs["fc_w"], np.float32)
    fc_b = np.asarray(inputs["fc_b"], np.float32)

    def wT(w, kc):     # [2, 1536, IN] -> [128, kc, 2, 12, 128]
        w = np.asarray(w, np.float32)
        a = w.transpose(2, 0, 1).reshape(kc, 128, 2, 12, 128)
        return np.ascontiguousarray(a.transpose(1, 0, 2, 3, 4)).astype(bft)

    WihT0 = wT(inputs["w_ih0"], 4)
    WhhT0 = wT(inputs["w_hh0"], 4)
    WihT1 = wT(inputs["w_ih1"], 8)
    WhhT1 = wT(inputs["w_hh1"], 4)

    # biasT[slot, layer, dir, g]
    biasT = np.zeros((16, 2, 2, 128), np.float32)
    for l, (bi, bh) in enumerate([
            (np.asarray(inputs["b_ih0"], np.float32),
             np.asarray(inputs["b_hh0"], np.float32)),
            (np.asarray(inputs["b_ih1"], np.float32),
             np.asarray(inputs["b_hh1"], np.float32))]):
        for d in range(2):
            rz = (bi[d, :1024] + bh[d, :1024]).reshape(8, 128)
            biasT[0:8, l, d, :] = rz
            biasT[8:12, l, d, :] = bh[d, 1024:].reshape(4, 128)
            biasT[12:16, l, d, :] = bi[d, 1024:].reshape(4, 128)
    biasT = biasT.astype(bft)

    sel = np.zeros((16, 16, W, BPC), np.float32)
    for s in range(16):
        sel[s, s] = 1.0
    sel = sel.astype(bft)

    fcw_pad = np.zeros((VPAD, 2 * H), np.float32)
    fcw_pad[:V] = fc_w
    fcb_pad = np.full((VPAD,), PADB, np.float32)
    fcb_pad[:V] = fc_b * (SW * SX)

    ones8 = np.zeros((1, 2, 128), np.float32)
    ones8[0, 0, :] = 1.0
    ones8 = ones8.astype(f8t)

    emb = emb_t[tgt]                      # [B, S, E]
    ctx4 = ctx.reshape(L, 2, B, H)        # [l, d, b, h]

    in_maps = []
    for c in range(NC_):
        bs = slice(BPC * c, BPC * (c + 1))
        emb_rows = emb[bs].transpose(1, 0, 2).reshape(R, E)   # row = 4t+b
        embT = np.ascontiguousarray(
            emb_rows.T.reshape(4, 128, R).transpose(1, 0, 2)).astype(bft)
        cc = ctx4[:, :, bs, :]                                # [l, d, 4, h]
        h0a = cc.transpose(3, 0, 1, 2).reshape(4, 128, L, 2, BPC)
        h0T = np.ascontiguousarray(
            h0a.transpose(1, 2, 3, 0, 4)).astype(bft)
        shard = fcw_pad[VS * c:VS * (c + 1)] * SW             # [VS, 1024]
        fcw8 = np.ascontiguousarray(
            shard.T.reshape(8, 128, VS).transpose(1, 0, 2)).astype(f8t)
        fcb8 = np.zeros((1, 4, 2, 1024), np.float32)
        fcb8[0, :, 0, :] = fcb_pad[VS * c:VS * (c + 1)].reshape(4, 1024)
        fcb8 = fcb8.astype(f8t)
        in_maps.append({
            "embT": embT, "h0T": h0T,
            "WihT0": WihT0, "WhhT0": WhhT0,
            "WihT1": WihT1, "WhhT1": WhhT1,
            "biasT": biasT, "sel16": sel,
            "fcw8": fcw8, "fcb8": fcb8, "ones8": ones8,
        })
    return in_maps


def _unshard(results):
    Lfull = np.concatenate(
        [results[c]["out"].astype(np.float32) for c in range(NC_)], axis=1)
    Lfull = Lfull[:, :V]                  # [4096, 32000]
    b = np.arange(B)[:, None]
    s = np.arange(S)[None, :]
    rows = (b // BPC) * R + BPC * s + (b % BPC)
    return Lfull[rows]                    # [B, S, V]


def kernel(**inputs):
    from concourse.bass_utils import run_bass_kernel_spmd
    nc = _get_nc()
    in_maps = _prep_inputs(inputs)
    res = run_bass_kernel_spmd(nc, in_maps, core_ids=list(range(NC_)))
    return _unshard(res.results)


# revision 35
# speedup vs baseline: 1.2534x; 1.0180x over previous
"""Trainium2 Bass kernel for nn_PlainDecoder (2-layer 2-dir GRU decoder +
vocab projection + log_softmax).

Sharding: data-parallel over batch (4 per core) for the scan; vocab-parallel
(4096-wide shard of padded 32768) for the logits.

Scan design (transposed orientation): all gate matmuls output
[128 gate-partitions, 4 batch] so the PE bill (prop. to output FREE size) is
tiny and h' is produced directly in lhsT (hidden-major) layout -- no PE
transposes.  Per (layer, dir) a windowed PSUM tile P holds, per step, 16
slots of 128 gates: [0:8]=r|z (preloaded with gi+bias), [8:12]=n-gh
(preloaded with b_hh_n), [12:16]=gi_n (+b_ih_n).  A bias matmul opens each
window bank (start=True), the windowed gi GEMM and the per-step Whh matmuls
accumulate on top (start=False).  Both layers run interleaved (layer 1 lags
LAG steps).  Everything bf16 into the PE, f32 in PSUM.

Logits: x2 (= layer-1 hist, bf16) is scaled to fp8 and AllGather'd; fc_w is
fp8.  Matmuls run in DoubleRow perf mode (K=256/instr, 0.5 cyc/row).  Per
(128-row block, 1024-vocab chunk): exp(l/256) with accumulated row sums and
a bf16 copy of l (frees PSUM fast); one AllReduce of partial sums per block;
out = l - ln(S) written bf16 (host converts to f32).
"""

import os
import sys
from contextlib import ExitStack

for _p in ("/opt/trn_rl_repo", "/root/.axon_site/_ro/trn_rl_repo"):
    if os.path.isdir(_p) and _p not in sys.path:
        sys.path.insert(0, _p)

import numpy as np  # noqa: E402
import ml_dtypes  # noqa: E402

V, E, H, L, B, S = 32000, 512, 512, 2, 32, 128
NC_ = 8                      # cores
BPC = B // NC_               # batches per core = 4
R = BPC * S                  # rows per core = 512 (s-major: row = 4*t + b)
VPAD = 32768
VS = VPAD // NC_             # vocab shard per core = 4096
W = 8                        # scan PSUM window (steps)
LAG = 11                     # layer-1 lag (steps)
NW = S // W                  # 16 windows
SW = 64.0                    # fc_w fp8 scale
SX = 4.0                     # x2 fp8 scale
SREC = 1.0 / (SW * SX)       # logits descale
PADB = -240.0                # pad-vocab scaled bias (e4m3 max finite)
NROW = NC_ * R               # 4096 global rows
NBLK = NROW // 128           # 32 row blocks

_BUILT = {}


def _build_nc(n_cores=NC_, sim=False):
    """Build the Bass program (same NEFF for all cores; per-core data
    differs).  sim=True replaces collectives with local DMAs so TimelineSim
    can run."""
    import concourse.bass as bass  # noqa: F401
    import concourse.mybir as mybir
    import concourse.tile as tile
    from concourse import bacc

    dt = mybir.dt
    f32 = dt.float32
    bf = dt.bfloat16
    f8 = dt.float8e4
    AF = mybir.ActivationFunctionType
    OP = mybir.AluOpType
    PM = mybir.MatmulPerfMode

    nc = bacc.Bacc("TRN2", target_bir_lowering=False, debug=False,
                   num_devices=n_cores)

    # ---------------- DRAM I/O ----------------
    embT = nc.dram_tensor("embT", [128, 4, R], bf, kind="ExternalInput")
    h0T = nc.dram_tensor("h0T", [128, 2, 2, 4, BPC], bf, kind="ExternalInput")
    WihT0 = nc.dram_tensor("WihT0", [128, 4, 2, 12, 128], bf,
                           kind="ExternalInput")
    WhhT0 = nc.dram_tensor("WhhT0", [128, 4, 2, 12, 128], bf,
                           kind="ExternalInput")
    WihT1 = nc.dram_tensor("WihT1", [128, 8, 2, 12, 128], bf,
                           kind="ExternalInput")
    WhhT1 = nc.dram_tensor("WhhT1", [128, 4, 2, 12, 128], bf,
                           kind="ExternalInput")
    biasT = nc.dram_tensor("biasT", [16, 2, 2, 128], bf, kind="ExternalInput")
    sel16 = nc.dram_tensor("sel16", [16, 16, W, BPC], bf, kind="ExternalInput")
    fcw8 = nc.dram_tensor("fcw8", [128, 8, VS], f8, kind="ExternalInput")
    fcb8 = nc.dram_tensor("fcb8", [1, 4, 2, 1024], f8, kind="ExternalInput")
    ones8 = nc.dram_tensor("ones8", [1, 2, 128], f8, kind="ExternalInput")

    out_d = nc.dram_tensor("out", [NROW, VS], bf, kind="ExternalOutput")

    # internal DRAM for collectives
    HR = R // 2
    agx_in = [nc.dram_tensor(f"agx_in{h}", [128, 8, HR], f8, kind="Internal")
              for h in range(2)]
    agx_out = [nc.dram_tensor(f"agx_out{h}", [n_cores * 128, 8, HR], f8,
                              kind="Internal", addr_space="Shared")
               for h in range(2)]
    NGRP = NBLK // 4          # 8 sum-collective groups of 4 row blocks
    ags_in = [nc.dram_tensor(f"ags_in{g}", [128, 4], f32, kind="Internal")
              for g in range(NGRP)]
    ags_out = [nc.dram_tensor(f"ags_out{g}", [128, 4], f32,
                              kind="Internal", addr_space="Shared")
               for g in range(NGRP)]
    rg = [list(range(n_cores))]

    with tile.TileContext(nc) as tc, ExitStack() as top:
        # logits weights pool (DMAs emitted after the scan weights so they
        # ride under the scan instead of delaying its start)
        fwpool = top.enter_context(tc.tile_pool(name="fw", bufs=1))
        fw = fwpool.tile([128, 8, VS], f8, tag="fw", name="fw")
        fb = fwpool.tile([1, 4, 2, 1024], f8, tag="fb", name="fb")
        on8 = fwpool.tile([1, 2, 128], f8, tag="on8", name="on8")

        # ---------------- scan phase ----------------
        with ExitStack() as scan_stack:
            wpool = scan_stack.enter_context(tc.tile_pool(name="wts", bufs=1))
            hpool = scan_stack.enter_context(tc.tile_pool(name="hist", bufs=1))
            cpool = scan_stack.enter_context(tc.tile_pool(name="chain", bufs=3))
            p0pool = scan_stack.enter_context(
                tc.tile_pool(name="P0", bufs=2, space="PSUM"))
            p1pool = scan_stack.enter_context(
                tc.tile_pool(name="P1", bufs=2, space="PSUM"))

            embT_sb = wpool.tile([128, 4, R], bf, tag="embT", name="embT_sb")
            nc.sync.dma_start(embT_sb[:], embT[:])
            h0_sb = wpool.tile([128, 2, 2, 4, BPC], bf, tag="h0", name="h0_sb")
            nc.sync.dma_start(h0_sb[:], h0T[:])
            wih0 = wpool.tile([128, 4, 2, 12, 128], bf, tag="wih0",
                              name="wih0")
            nc.sync.dma_start(wih0[:], WihT0[:])
            whh0 = wpool.tile([128, 4, 2, 12, 128], bf, tag="whh0",
                              name="whh0")
            nc.sync.dma_start(whh0[:], WhhT0[:])
            wih1 = wpool.tile([128, 8, 2, 12, 128], bf, tag="wih1",
                              name="wih1")
            nc.sync.dma_start(wih1[:], WihT1[:])
            whh1 = wpool.tile([128, 4, 2, 12, 128], bf, tag="whh1",
                              name="whh1")
            nc.sync.dma_start(whh1[:], WhhT1[:])
            bias_sb = wpool.tile([16, 2, 2, 128], bf, tag="bias",
                                 name="bias_sb")
            nc.sync.dma_start(bias_sb[:], biasT[:])
            sel_sb = wpool.tile([16, 16, W, BPC], bf, tag="sel", name="sel_sb")
            nc.sync.dma_start(sel_sb[:], sel16[:])
            nc.sync.dma_start(fw[:], fcw8[:])
            nc.sync.dma_start(fb[:], fcb8[:])
            nc.sync.dma_start(on8[:], ones8[:])

            # hist layout: [128 h-part, dir, kchunk, row(=4t+b)]
            hist = [hpool.tile([128, 2, 4, R], bf, tag=f"hist{l}",
                               name=f"hist{l}") for l in range(2)]
            ppool = [p0pool, p1pool]
            wih = [wih0, wih1]
            whh = [whh0, whh1]
            kc_ih = [4, 8]
            pwin = [{}, {}]       # (layer, window) -> PSUM tile
            gwin = [{}, {}]       # (layer, window) -> SBUF gi_n tile

            def gi_window(l, w, dirs=(0, 1)):
                """Bias opener + gi GEMM for window w of layer l (emitted
                per dir so late windows can be sliced across iterations).
                P layout: [128, dir, slot16, W, BPC]."""
                if 0 in dirs:
                    P = ppool[l].tile([128, 2, 16, W, BPC], f32, tag=f"P{l}",
                                      name=f"P{l}w{w}")
                    pwin[l][w] = P
                    gin = cpool.tile([128, 2, 4, W, BPC], f32, tag=f"gin{l}",
                                     bufs=2, name=f"gin{l}")
                    gwin[l][w] = gin
                P = pwin[l][w]
                gin = gwin[l][w]
                rows = slice(BPC * W * w, BPC * W * (w + 1))
                for d in dirs:
                    nc.tensor.matmul(P[:, d], bias_sb[:, l, d, :], sel_sb[:],
                                     start=True, stop=False,
                                     skip_group_check=True)
                    for sl in range(12):
                        slot = sl if sl < 8 else sl + 4
                        for k in range(kc_ih[l]):
                            if l == 0:
                                rhs = embT_sb[:, k, rows]
                            else:
                                rhs = hist[0][:, k // 4, k % 4, rows]
                            nc.tensor.matmul(
                                P[:, d, slot], wih[l][:, k, d, sl, :], rhs,
                                start=False,
                                stop=(slot >= 12 and k == kc_ih[l] - 1),
                                skip_group_check=True)
                    # stage gi_n into SBUF so the in-loop add is SBUF-only
                    nc.vector.tensor_copy(gin[:, d], P[:, d, 12:16, :, :])

            def scan_step(l, t):
                """Whh matmuls + GRU cell chain for step t of layer l."""
                P = pwin[l][t // W]
                gin = gwin[l][t // W]
                tw = t % W
                if t == 0:
                    hp = h0_sb[:, l]                      # [128, 2, 4, BPC]
                else:
                    hp = hist[l][:, :, :, BPC * (t - 1):BPC * t]
                # k-major, r|z slices first: k0/k1 matmuls wait only on the
                # first half of the h' tail, and the sigmoid (head of the
                # serial chain) doesn't wait on the n-gh matmuls
                for j0, j1 in ((0, 8), (8, 12)):
                    for k in range(4):
                        for d in range(2):
                            for j in range(j0, j1):
                                nc.tensor.matmul(
                                    P[:, d, j, tw, :], whh[l][:, k, d, j, :],
                                    hp[:, d, k, :], start=False,
                                    stop=(k == 3), skip_group_check=True)
                # GRU cell chain (both dirs in one op each):
                #   h' = n*(1-z) + z*hprev ; q=z*hprev and u=1-z leave the
                #   serial path right after the sigmoid
                rzs = cpool.tile([128, 2, 8, BPC], f32, tag=f"rzs{l}",
                                 name=f"rzs{l}")
                nc.scalar.activation(rzs[:], P[:, :, 0:8, tw, :], AF.Sigmoid)
                n1 = cpool.tile([128, 2, 4, BPC], f32, tag=f"n1{l}",
                                name=f"n1{l}")
                nc.vector.tensor_mul(n1[:], P[:, :, 8:12, tw, :],
                                     rzs[:, :, 0:4, :])
                nc.vector.tensor_add(n1[:], n1[:], gin[:, :, :, tw, :])
                q = cpool.tile([128, 2, 4, BPC], f32, tag=f"q{l}",
                               name=f"q{l}")
                nc.vector.tensor_mul(q[:], rzs[:, :, 4:8, :], hp[:])
                u = cpool.tile([128, 2, 4, BPC], f32, tag=f"u{l}",
                               name=f"u{l}")
                nc.vector.tensor_scalar(u[:], rzs[:, :, 4:8, :], -1.0, 1.0,
                                        OP.mult, OP.add)
                nt = cpool.tile([128, 2, 4, BPC], f32, tag=f"nt{l}",
                                name=f"nt{l}")
                nc.scalar.activation(nt[:], n1[:], AF.Tanh)
                # tail on Pool (no ack latency, SBUF-only operands), split per
                # k-half so next-step k0/k1 matmuls start before k2/k3 finish
                d1 = cpool.tile([128, 2, 4, BPC], f32, tag=f"d1{l}",
                                name=f"d1{l}")
                for ks in (slice(0, 2), slice(2, 4)):
                    nc.gpsimd.tensor_mul(d1[:, :, ks], nt[:, :, ks],
                                         u[:, :, ks])
                    nc.gpsimd.tensor_add(
                        hist[l][:, :, ks, BPC * t:BPC * (t + 1)],
                        d1[:, :, ks], q[:, :, ks])

            def ship_x2(h):
                rows = slice(HR * h, HR * (h + 1))
                x8 = cpool.tile([128, 2, 4, HR], f8, tag="x8", bufs=2,
                                name="x8")
                nc.vector.tensor_scalar_mul(x8[:], hist[1][:, :, :, rows], SX)
                nc.sync.dma_start(agx_in[h][:],
                                  x8[:].rearrange("p d k r -> p (d k) r"))
                if sim:
                    nc.sync.dma_start(agx_out[h][0:128], agx_in[h][:])
                else:
                    nc.gpsimd.collective_compute(
                        "AllGather", OP.bypass, replica_groups=rg,
                        ins=[agx_in[h][:].opt()], outs=[agx_out[h][:].opt()])

            gi_window(0, 0)
            for it in range(S + LAG):
                if it == S // 2 + LAG + 1:
                    ship_x2(0)        # L1 rows 0..255 done; gather them now
                if it % W == 5 and (it + 3) // W < NW:
                    w = (it + 3) // W
                    gi_window(0, w, dirs=(0, 1) if w < 14 else (0,))
                if it % W == 6 and (it + 2) // W >= 14 and (it + 2) // W < NW:
                    gi_window(0, (it + 2) // W, dirs=(1,))
                if it % W == 1 and it >= 9 and (it - 9) // W < NW:
                    w = (it - 9) // W
                    gi_window(1, w, dirs=(0, 1) if w < 12 else (0,))
                if it % W == 2 and it >= 10 and (it - 10) // W >= 12 and (it - 10) // W < NW:
                    gi_window(1, (it - 10) // W, dirs=(1,))
                if it < S:
                    scan_step(0, it)
                t1 = it - LAG
                if 0 <= t1 < S:
                    scan_step(1, t1)

            ship_x2(1)

        # ---------------- logits + log_softmax ----------------
        with (
            tc.tile_pool(name="lt", bufs=3) as ltpool,
            tc.tile_pool(name="lps", bufs=4, space="PSUM") as lpspool,
            tc.tile_pool(name="lsc", bufs=2) as lscpool,
        ):
            x2g = ltpool.tile([128, 8, 8, R], f8, tag="x2g", bufs=1,
                              name="x2g")
            # per-source-core, per-half chunks: contiguous DMAs; the first
            # half is gathered mid-scan, so early row blocks start sooner
            for c in range(8):
                for h in range(2):
                    nc.sync.dma_start(
                        x2g[:, :, c, HR * h:HR * (h + 1)],
                        agx_out[h][128 * c:128 * (c + 1), :, :])

            def block_mm(rb, lb, sgrp, rb4):
                """Matmuls + exp/sums + bf16-l copy for one 128-row block.
                All 40 matmuls are emitted back-to-back (keeps the PE
                p-state ramped); exp/copies follow."""
                csrc, r0 = rb // BPC, (rb % BPC) * 128
                srb = lscpool.tile([128, 4], f32, tag="srb", bufs=3,
                                   name="srb")
                def vq_mms(vq):
                    P = lpspool.tile([128, 1024], f32, tag="lp", name="lp")
                    # matmuls per 512-wide half: a PSUM matmul target must
                    # stay within one 2KB bank
                    for hh in range(2):
                        v0 = 1024 * vq + 512 * hh
                        nc.tensor.matmul(P[:, 512 * hh:512 * (hh + 1)],
                                         on8[:], fb[0:1, vq, :,
                                                    512 * hh:512 * (hh + 1)],
                                         start=True, stop=False,
                                         perf_mode=PM.DoubleRow,
                                         skip_group_check=True)
                        for c2 in range(4):
                            nc.tensor.matmul(
                                P[:, 512 * hh:512 * (hh + 1)],
                                x2g[:, 2 * c2:2 * c2 + 2, csrc, r0:r0 + 128],
                                fw[:, 2 * c2:2 * c2 + 2, v0:v0 + 512],
                                start=False, stop=(c2 == 3),
                                perf_mode=PM.DoubleRow, skip_group_check=True)
                    return P

                def vq_post(vq, P):
                    eb = lscpool.tile([128, 1024], bf, tag="eb", bufs=3,
                                      name="eb")
                    nc.scalar.activation(eb[:], P[:], AF.Exp, scale=SREC,
                                         accum_out=srb[:, vq:vq + 1])
                    # bf16 copy of l frees the PSUM bank quickly
                    nc.vector.tensor_scalar_mul(lb[:, vq, :], P[:], SREC)

                # software-pipelined: chunk vq's exp/copy emitted after chunk
                # vq+1's matmuls so the PE stream never waits on PSUM reuse
                Ps = [vq_mms(0)]
                for vq in range(1, 4):
                    Ps.append(vq_mms(vq))
                    vq_post(vq - 1, Ps[vq - 1])
                vq_post(3, Ps[3])
                nc.vector.tensor_reduce(sgrp[:, rb4:rb4 + 1], srb[:],
                                        axis=mybir.AxisListType.X, op=OP.add)

            for g in range(NGRP):
                rbs = list(range(4 * g, 4 * g + 4))
                lbs = []
                obs = []
                sgrp = lscpool.tile([128, 4], f32, tag="sgrp", bufs=3,
                                    name="sgrp")
                for rb4 in range(4):
                    lb = ltpool.tile([128, 4, 1024], bf, tag="lb", bufs=6,
                                     name="lb")
                    block_mm(rbs[rb4], lb, sgrp, rb4)
                    lbs.append(lb)
                nc.sync.dma_start(ags_in[g][:], sgrp[:])
                if sim:
                    nc.sync.dma_start(ags_out[g][:], ags_in[g][:])
                else:
                    nc.gpsimd.collective_compute(
                        "AllReduce", OP.add, replica_groups=rg,
                        ins=[ags_in[g][:].opt()],
                        outs=[ags_out[g][:].opt()])
                sg = lscpool.tile([128, 4], f32, tag="sg", name="sg")
                nc.sync.dma_start(sg[:], ags_out[g][:])
                lnS = lscpool.tile([128, 4], f32, tag="lnS", name="lnS")
                nc.scalar.activation(lnS[:], sg[:], AF.Ln)
                for rb4 in range(4):
                    ob = ltpool.tile([128, VS], bf, tag="ob", bufs=4,
                                     name="ob")
                    for vq in range(4):
                        # balance the subtract across Pool (2) / DVE (2);
                        # ACT is the logits bottleneck, keep it clear
                        eng = nc.gpsimd if vq < 2 else nc.vector
                        eng.tensor_scalar_sub(
                            ob[:, 1024 * vq:1024 * (vq + 1)],
                            lbs[rb4][:, vq, :], lnS[:, rb4:rb4 + 1])
                    obs.append(ob)
                for rb4 in range(4):
                    rb = rbs[rb4]
                    nc.sync.dma_start(out_d[128 * rb:128 * (rb + 1), :],
                                      obs[rb4][:])

    nc.compile()
    return nc


def _get_nc():
    if "nc" not in _BUILT:
        _BUILT["nc"] = _build_nc()
    return _BUILT["nc"]


def _prep_inputs(inputs):
    """Host-side shard + relayout. Returns in_maps for 8 cores."""
    bft = ml_dtypes.bfloat16
    f8t = ml_dtypes.float8_e4m3

    tgt = np.asarray(inputs["target"])
    ctx = np.asarray(inputs["context"], np.float32)
    emb_t = np.asarray(inputs["embed_table"], np.float32)
    fc_w = np.asarray(inputs["fc_w"], np.float32)
    fc_b = np.asarray(inputs["fc_b"], np.float32)

    def wT(w, kc):     # [2, 1536, IN] -> [128, kc, 2, 12, 128]
        w = np.asarray(w, np.float32)
        a = w.transpose(2, 0, 1).reshape(kc, 128, 2, 12, 128)
        return np.ascontiguousarray(a.transpose(1, 0, 2, 3, 4)).astype(bft)

    WihT0 = wT(inputs["w_ih0"], 4)
    WhhT0 = wT(inputs["w_hh0"], 4)
    WihT1 = wT(inputs["w_ih1"], 8)
    WhhT1 = wT(inputs["w_hh1"], 4)

    # biasT[slot, layer, dir, g]
    biasT = np.zeros((16, 2, 2, 128), np.float32)
    for l, (bi, bh) in enumerate([
            (np.asarray(inputs["b_ih0"], np.float32),
             np.asarray(inputs["b_hh0"], np.float32)),
            (np.asarray(inputs["b_ih1"], np.float32),
             np.asarray(inputs["b_hh1"], np.float32))]):
        for d in range(2):
            rz = (bi[d, :1024] + bh[d, :1024]).reshape(8, 128)
            biasT[0:8, l, d, :] = rz
            biasT[8:12, l, d, :] = bh[d, 1024:].reshape(4, 128)
            biasT[12:16, l, d, :] = bi[d, 1024:].reshape(4, 128)
    biasT = biasT.astype(bft)

    sel = np.zeros((16, 16, W, BPC), np.float32)
    for s in range(16):
        sel[s, s] = 1.0
    sel = sel.astype(bft)

    fcw_pad = np.zeros((VPAD, 2 * H), np.float32)
    fcw_pad[:V] = fc_w
    fcb_pad = np.full((VPAD,), PADB, np.float32)
    fcb_pad[:V] = fc_b * (SW * SX)

    ones8 = np.zeros((1, 2, 128), np.float32)
    ones8[0, 0, :] = 1.0
    ones8 = ones8.astype(f8t)

    emb = emb_t[tgt]                      # [B, S, E]
    ctx4 = ctx.reshape(L, 2, B, H)        # [l, d, b, h]

    in_maps = []
    for c in range(NC_):
        bs = slice(BPC * c, BPC * (c + 1))
        emb_rows = emb[bs].transpose(1, 0, 2).reshape(R, E)   # row = 4t+b
        embT = np.ascontiguousarray(
            emb_rows.T.reshape(4, 128, R).transpose(1, 0, 2)).astype(bft)
        cc = ctx4[:, :, bs, :]                                # [l, d, 4, h]
        h0a = cc.transpose(3, 0, 1, 2).reshape(4, 128, L, 2, BPC)
        h0T = np.ascontiguousarray(
            h0a.transpose(1, 2, 3, 0, 4)).astype(bft)
        shard = fcw_pad[VS * c:VS * (c + 1)] * SW             # [VS, 1024]
        fcw8 = np.ascontiguousarray(
            shard.T.reshape(8, 128, VS).transpose(1, 0, 2)).astype(f8t)
        fcb8 = np.zeros((1, 4, 2, 1024), np.float32)
        fcb8[0, :, 0, :] = fcb_pad[VS * c:VS * (c + 1)].reshape(4, 1024)
        fcb8 = fcb8.astype(f8t)
        in_maps.append({
            "embT": embT, "h0T": h0T,
            "WihT0": WihT0, "WhhT0": WhhT0,
            "WihT1": WihT1, "WhhT1": WhhT1,
            "biasT": biasT, "sel16": sel,
            "fcw8": fcw8, "fcb8": fcb8, "ones8": ones8,
        })
    return in_maps


def _unshard(results):
    Lfull = np.concatenate(
        [results[c]["out"].astype(np.float32) for c in range(NC_)], axis=1)
    Lfull = Lfull[:, :V]                  # [4096, 32000]
    b = np.arange(B)[:, None]
    s = np.arange(S)[None, :]
    rows = (b // BPC) * R + BPC * s + (b % BPC)
    return Lfull[rows]                    # [B, S, V]


def kernel(**inputs):
    from concourse.bass_utils import run_bass_kernel_spmd
    nc = _get_nc()
    in_maps = _prep_inputs(inputs)
    res = run_bass_kernel_spmd(nc, in_maps, core_ids=list(range(NC_)))
    return _unshard(res.results)


# revision 41
# speedup vs baseline: 1.2550x; 1.0013x over previous
"""Trainium2 Bass kernel for nn_PlainDecoder (2-layer 2-dir GRU decoder +
vocab projection + log_softmax).

Sharding: data-parallel over batch (4 per core) for the scan; vocab-parallel
(4096-wide shard of padded 32768) for the logits.

Scan design (transposed orientation): all gate matmuls output
[128 gate-partitions, 4 batch] so the PE bill (prop. to output FREE size) is
tiny and h' is produced directly in lhsT (hidden-major) layout -- no PE
transposes.  Per (layer, dir) a windowed PSUM tile P holds, per step, 16
slots of 128 gates: [0:8]=r|z (preloaded with gi+bias), [8:12]=n-gh
(preloaded with b_hh_n), [12:16]=gi_n (+b_ih_n).  A bias matmul opens each
window bank (start=True), the windowed gi GEMM and the per-step Whh matmuls
accumulate on top (start=False).  Both layers run interleaved (layer 1 lags
LAG steps).  Everything bf16 into the PE, f32 in PSUM.

Logits: x2 (= layer-1 hist, bf16) is scaled to fp8 and AllGather'd; fc_w is
fp8.  Matmuls run in DoubleRow perf mode (K=256/instr, 0.5 cyc/row).  Per
(128-row block, 1024-vocab chunk): exp(l/256) with accumulated row sums and
a bf16 copy of l (frees PSUM fast); one AllReduce of partial sums per block;
out = l - ln(S) written bf16 (host converts to f32).
"""

import os
import sys
from contextlib import ExitStack

for _p in ("/opt/trn_rl_repo", "/root/.axon_site/_ro/trn_rl_repo"):
    if os.path.isdir(_p) and _p not in sys.path:
        sys.path.insert(0, _p)

import numpy as np  # noqa: E402
import ml_dtypes  # noqa: E402

V, E, H, L, B, S = 32000, 512, 512, 2, 32, 128
NC_ = 8                      # cores
BPC = B // NC_               # batches per core = 4
R = BPC * S                  # rows per core = 512 (s-major: row = 4*t + b)
VPAD = 32768
VS = VPAD // NC_             # vocab shard per core = 4096
W = 8                        # scan PSUM window (steps)
LAG = 11                     # layer-1 lag (steps)
NW = S // W                  # 16 windows
SW = 64.0                    # fc_w fp8 scale
SX = 4.0                     # x2 fp8 scale
SREC = 1.0 / (SW * SX)       # logits descale
PADB = -240.0                # pad-vocab scaled bias (e4m3 max finite)
NROW = NC_ * R               # 4096 global rows
NBLK = NROW // 128           # 32 row blocks

_BUILT = {}


def _build_nc(n_cores=NC_, sim=False):
    """Build the Bass program (same NEFF for all cores; per-core data
    differs).  sim=True replaces collectives with local DMAs so TimelineSim
    can run."""
    import concourse.bass as bass  # noqa: F401
    import concourse.mybir as mybir
    import concourse.tile as tile
    from concourse import bacc

    dt = mybir.dt
    f32 = dt.float32
    bf = dt.bfloat16
    f8 = dt.float8e4
    AF = mybir.ActivationFunctionType
    OP = mybir.AluOpType
    PM = mybir.MatmulPerfMode

    nc = bacc.Bacc("TRN2", target_bir_lowering=False, debug=False,
                   num_devices=n_cores)

    # ---------------- DRAM I/O ----------------
    embT = nc.dram_tensor("embT", [128, 4, R], bf, kind="ExternalInput")
    h0T = nc.dram_tensor("h0T", [128, 2, 2, 4, BPC], bf, kind="ExternalInput")
    WihT0 = nc.dram_tensor("WihT0", [128, 4, 2, 12, 128], bf,
                           kind="ExternalInput")
    WhhT0 = nc.dram_tensor("WhhT0", [128, 4, 2, 12, 128], bf,
                           kind="ExternalInput")
    WihT1 = nc.dram_tensor("WihT1", [128, 8, 2, 12, 128], bf,
                           kind="ExternalInput")
    WhhT1 = nc.dram_tensor("WhhT1", [128, 4, 2, 12, 128], bf,
                           kind="ExternalInput")
    biasT = nc.dram_tensor("biasT", [16, 2, 2, 128], bf, kind="ExternalInput")
    sel16 = nc.dram_tensor("sel16", [16, 16, W, BPC], bf, kind="ExternalInput")
    fcw8 = nc.dram_tensor("fcw8", [128, 8, VS], f8, kind="ExternalInput")
    fcb8 = nc.dram_tensor("fcb8", [1, 4, 2, 1024], f8, kind="ExternalInput")
    ones8 = nc.dram_tensor("ones8", [1, 2, 128], f8, kind="ExternalInput")

    out_d = nc.dram_tensor("out", [NROW, VS], bf, kind="ExternalOutput")

    # internal DRAM for collectives
    HR = R // 2
    agx_in = [nc.dram_tensor(f"agx_in{h}", [128, 8, HR], f8, kind="Internal")
              for h in range(2)]
    agx_out = [nc.dram_tensor(f"agx_out{h}", [n_cores * 128, 8, HR], f8,
                              kind="Internal", addr_space="Shared")
               for h in range(2)]
    NGRP = NBLK // 4          # 8 sum-collective groups of 4 row blocks
    ags_in = [nc.dram_tensor(f"ags_in{g}", [128, 4], f32, kind="Internal")
              for g in range(NGRP)]
    ags_out = [nc.dram_tensor(f"ags_out{g}", [128, 4], f32,
                              kind="Internal", addr_space="Shared")
               for g in range(NGRP)]
    rg = [list(range(n_cores))]

    with tile.TileContext(nc) as tc, ExitStack() as top:
        # logits weights pool (DMAs emitted after the scan weights so they
        # ride under the scan instead of delaying its start)
        fwpool = top.enter_context(tc.tile_pool(name="fw", bufs=1))
        fw = fwpool.tile([128, 8, VS], f8, tag="fw", name="fw")
        fb = fwpool.tile([1, 4, 2, 1024], f8, tag="fb", name="fb")
        on8 = fwpool.tile([1, 2, 128], f8, tag="on8", name="on8")

        # ---------------- scan phase ----------------
        with ExitStack() as scan_stack:
            wpool = scan_stack.enter_context(tc.tile_pool(name="wts", bufs=1))
            hpool = scan_stack.enter_context(tc.tile_pool(name="hist", bufs=1))
            cpool = scan_stack.enter_context(tc.tile_pool(name="chain", bufs=3))
            p0pool = scan_stack.enter_context(
                tc.tile_pool(name="P0", bufs=2, space="PSUM"))
            p1pool = scan_stack.enter_context(
                tc.tile_pool(name="P1", bufs=2, space="PSUM"))

            embT_sb = wpool.tile([128, 4, R], bf, tag="embT", name="embT_sb")
            nc.sync.dma_start(embT_sb[:], embT[:])
            h0_sb = wpool.tile([128, 2, 2, 4, BPC], bf, tag="h0", name="h0_sb")
            nc.sync.dma_start(h0_sb[:], h0T[:])
            wih0 = wpool.tile([128, 4, 2, 12, 128], bf, tag="wih0",
                              name="wih0")
            nc.sync.dma_start(wih0[:], WihT0[:])
            whh0 = wpool.tile([128, 4, 2, 12, 128], bf, tag="whh0",
                              name="whh0")
            nc.sync.dma_start(whh0[:], WhhT0[:])
            wih1 = wpool.tile([128, 8, 2, 12, 128], bf, tag="wih1",
                              name="wih1")
            nc.sync.dma_start(wih1[:], WihT1[:])
            whh1 = wpool.tile([128, 4, 2, 12, 128], bf, tag="whh1",
                              name="whh1")
            nc.sync.dma_start(whh1[:], WhhT1[:])
            bias_sb = wpool.tile([16, 2, 2, 128], bf, tag="bias",
                                 name="bias_sb")
            nc.sync.dma_start(bias_sb[:], biasT[:])
            sel_sb = wpool.tile([16, 16, W, BPC], bf, tag="sel", name="sel_sb")
            nc.sync.dma_start(sel_sb[:], sel16[:])
            nc.sync.dma_start(fw[:], fcw8[:])
            nc.sync.dma_start(fb[:], fcb8[:])
            nc.sync.dma_start(on8[:], ones8[:])

            # hist layout: [128 h-part, dir, kchunk, row(=4t+b)]
            hist = [hpool.tile([128, 2, 4, R], bf, tag=f"hist{l}",
                               name=f"hist{l}") for l in range(2)]
            ppool = [p0pool, p1pool]
            wih = [wih0, wih1]
            whh = [whh0, whh1]
            kc_ih = [4, 8]
            pwin = [{}, {}]       # (layer, window) -> PSUM tile
            gwin = [{}, {}]       # (layer, window) -> SBUF gi_n tile

            def gi_window(l, w, dirs=(0, 1)):
                """Bias opener + gi GEMM for window w of layer l (emitted
                per dir so late windows can be sliced across iterations).
                P layout: [128, dir, slot16, W, BPC]."""
                if 0 in dirs:
                    P = ppool[l].tile([128, 2, 16, W, BPC], f32, tag=f"P{l}",
                                      name=f"P{l}w{w}")
                    pwin[l][w] = P
                    gin = cpool.tile([128, 2, 4, W, BPC], f32, tag=f"gin{l}",
                                     bufs=2, name=f"gin{l}")
                    gwin[l][w] = gin
                P = pwin[l][w]
                gin = gwin[l][w]
                rows = slice(BPC * W * w, BPC * W * (w + 1))
                for d in dirs:
                    nc.tensor.matmul(P[:, d], bias_sb[:, l, d, :], sel_sb[:],
                                     start=True, stop=False,
                                     skip_group_check=True)
                    for sl in range(12):
                        slot = sl if sl < 8 else sl + 4
                        for k in range(kc_ih[l]):
                            if l == 0:
                                rhs = embT_sb[:, k, rows]
                            else:
                                rhs = hist[0][:, k // 4, k % 4, rows]
                            nc.tensor.matmul(
                                P[:, d, slot], wih[l][:, k, d, sl, :], rhs,
                                start=False,
                                stop=(slot >= 12 and k == kc_ih[l] - 1),
                                skip_group_check=True)
                    # stage gi_n into SBUF so the in-loop add is SBUF-only
                    nc.vector.tensor_copy(gin[:, d], P[:, d, 12:16, :, :])

            def scan_step(l, t):
                """Whh matmuls + GRU cell chain for step t of layer l."""
                P = pwin[l][t // W]
                gin = gwin[l][t // W]
                tw = t % W
                if t == 0:
                    hp = h0_sb[:, l]                      # [128, 2, 4, BPC]
                else:
                    hp = hist[l][:, :, :, BPC * (t - 1):BPC * t]
                # k-major, r|z slices first: k0/k1 matmuls wait only on the
                # first half of the h' tail, and the sigmoid (head of the
                # serial chain) doesn't wait on the n-gh matmuls
                for j0, j1 in ((0, 8), (8, 12)):
                    for k in range(4):
                        for d in range(2):
                            for j in range(j0, j1):
                                nc.tensor.matmul(
                                    P[:, d, j, tw, :], whh[l][:, k, d, j, :],
                                    hp[:, d, k, :], start=False,
                                    stop=(k == 3), skip_group_check=True)
                # GRU cell chain (both dirs in one op each):
                #   h' = n*(1-z) + z*hprev ; q=z*hprev and u=1-z leave the
                #   serial path right after the sigmoid
                rzs = cpool.tile([128, 2, 8, BPC], f32, tag=f"rzs{l}",
                                 name=f"rzs{l}")
                nc.scalar.activation(rzs[:], P[:, :, 0:8, tw, :], AF.Sigmoid)
                n1 = cpool.tile([128, 2, 4, BPC], f32, tag=f"n1{l}",
                                name=f"n1{l}")
                nc.vector.tensor_mul(n1[:], P[:, :, 8:12, tw, :],
                                     rzs[:, :, 0:4, :])
                nc.vector.tensor_add(n1[:], n1[:], gin[:, :, :, tw, :])
                q = cpool.tile([128, 2, 4, BPC], f32, tag=f"q{l}",
                               name=f"q{l}")
                nc.vector.tensor_mul(q[:], rzs[:, :, 4:8, :], hp[:])
                u = cpool.tile([128, 2, 4, BPC], f32, tag=f"u{l}",
                               name=f"u{l}")
                nc.vector.tensor_scalar(u[:], rzs[:, :, 4:8, :], -1.0, 1.0,
                                        OP.mult, OP.add)
                nt = cpool.tile([128, 2, 4, BPC], f32, tag=f"nt{l}",
                                name=f"nt{l}")
                nc.scalar.activation(nt[:], n1[:], AF.Tanh)
                # tail on Pool (no ack latency, SBUF-only operands), split per
                # k-half so next-step k0/k1 matmuls start before k2/k3 finish
                d1 = cpool.tile([128, 2, 4, BPC], f32, tag=f"d1{l}",
                                name=f"d1{l}")
                for ks in (slice(0, 2), slice(2, 4)):
                    nc.gpsimd.tensor_mul(d1[:, :, ks], nt[:, :, ks],
                                         u[:, :, ks])
                    nc.gpsimd.tensor_add(
                        hist[l][:, :, ks, BPC * t:BPC * (t + 1)],
                        d1[:, :, ks], q[:, :, ks])

            def ship_x2(h):
                rows = slice(HR * h, HR * (h + 1))
                x8 = cpool.tile([128, 2, 4, HR], f8, tag="x8", bufs=2,
                                name="x8")
                nc.vector.tensor_scalar_mul(x8[:], hist[1][:, :, :, rows], SX)
                nc.sync.dma_start(agx_in[h][:],
                                  x8[:].rearrange("p d k r -> p (d k) r"))
                if sim:
                    nc.sync.dma_start(agx_out[h][0:128], agx_in[h][:])
                else:
                    nc.gpsimd.collective_compute(
                        "AllGather", OP.bypass, replica_groups=rg,
                        ins=[agx_in[h][:].opt()], outs=[agx_out[h][:].opt()])

            gi_window(0, 0)
            for it in range(S + LAG):
                if it == S // 2 + LAG + 1:
                    ship_x2(0)        # L1 rows 0..255 done; gather them now
                if it % W == 5 and (it + 3) // W < NW:
                    gi_window(0, (it + 3) // W, dirs=(0,))
                if it % W == 6 and (it + 2) // W < NW:
                    gi_window(0, (it + 2) // W, dirs=(1,))
                if it % W == 1 and it >= 9 and (it - 9) // W < NW:
                    gi_window(1, (it - 9) // W, dirs=(0,))
                if it % W == 2 and it >= 10 and (it - 10) // W < NW:
                    gi_window(1, (it - 10) // W, dirs=(1,))
                if it < S:
                    scan_step(0, it)
                t1 = it - LAG
                if 0 <= t1 < S:
                    scan_step(1, t1)

            ship_x2(1)

        # ---------------- logits + log_softmax ----------------
        with (
            tc.tile_pool(name="lt", bufs=3) as ltpool,
            tc.tile_pool(name="lps", bufs=4, space="PSUM") as lpspool,
            tc.tile_pool(name="lsc", bufs=2) as lscpool,
        ):
            x2g = ltpool.tile([128, 8, 8, R], f8, tag="x2g", bufs=1,
                              name="x2g")
            # per-source-core, per-half chunks: contiguous DMAs; the first
            # half is gathered mid-scan, so early row blocks start sooner
            for c in range(8):
                for h in range(2):
                    nc.sync.dma_start(
                        x2g[:, :, c, HR * h:HR * (h + 1)],
                        agx_out[h][128 * c:128 * (c + 1), :, :])

            def block_mm(rb, lb, sgrp, rb4):
                """Matmuls + exp/sums + bf16-l copy for one 128-row block.
                All 40 matmuls are emitted back-to-back (keeps the PE
                p-state ramped); exp/copies follow."""
                csrc, r0 = rb // BPC, (rb % BPC) * 128
                srb = lscpool.tile([128, 4], f32, tag="srb", bufs=3,
                                   name="srb")
                def vq_mms(vq):
                    P = lpspool.tile([128, 1024], f32, tag="lp", name="lp")
                    # matmuls per 512-wide half: a PSUM matmul target must
                    # stay within one 2KB bank
                    for hh in range(2):
                        v0 = 1024 * vq + 512 * hh
                        nc.tensor.matmul(P[:, 512 * hh:512 * (hh + 1)],
                                         on8[:], fb[0:1, vq, :,
                                                    512 * hh:512 * (hh + 1)],
                                         start=True, stop=False,
                                         perf_mode=PM.DoubleRow,
                                         skip_group_check=True)
                        for c2 in range(4):
                            nc.tensor.matmul(
                                P[:, 512 * hh:512 * (hh + 1)],
                                x2g[:, 2 * c2:2 * c2 + 2, csrc, r0:r0 + 128],
                                fw[:, 2 * c2:2 * c2 + 2, v0:v0 + 512],
                                start=False, stop=(c2 == 3),
                                perf_mode=PM.DoubleRow, skip_group_check=True)
                    return P

                def vq_post(vq, P):
                    eb = lscpool.tile([128, 1024], bf, tag="eb", bufs=3,
                                      name="eb")
                    nc.scalar.activation(eb[:], P[:], AF.Exp, scale=SREC,
                                         accum_out=srb[:, vq:vq + 1])
                    # bf16 copy of l frees the PSUM bank quickly
                    nc.vector.tensor_scalar_mul(lb[:, vq, :], P[:], SREC)

                # software-pipelined: chunk vq's exp/copy emitted after chunk
                # vq+1's matmuls so the PE stream never waits on PSUM reuse
                Ps = [vq_mms(0)]
                for vq in range(1, 4):
                    Ps.append(vq_mms(vq))
                    vq_post(vq - 1, Ps[vq - 1])
                vq_post(3, Ps[3])
                nc.vector.tensor_reduce(sgrp[:, rb4:rb4 + 1], srb[:],
                                        axis=mybir.AxisListType.X, op=OP.add)

            for g in range(NGRP):
                rbs = list(range(4 * g, 4 * g + 4))
                lbs = []
                obs = []
                sgrp = lscpool.tile([128, 4], f32, tag="sgrp", bufs=3,
                                    name="sgrp")
                for rb4 in range(4):
                    lb = ltpool.tile([128, 4, 1024], bf, tag="lb", bufs=6,
                                     name="lb")
                    block_mm(rbs[rb4], lb, sgrp, rb4)
                    lbs.append(lb)
                nc.sync.dma_start(ags_in[g][:], sgrp[:])
                if sim:
                    nc.sync.dma_start(ags_out[g][:], ags_in[g][:])
                else:
                    nc.gpsimd.collective_compute(
                        "AllReduce", OP.add, replica_groups=rg,
                        ins=[ags_in[g][:].opt()],
                        outs=[ags_out[g][:].opt()])
                sg = lscpool.tile([128, 4], f32, tag="sg", name="sg")
                nc.sync.dma_start(sg[:], ags_out[g][:])
                lnS = lscpool.tile([128, 4], f32, tag="lnS", name="lnS")
                nc.scalar.activation(lnS[:], sg[:], AF.Ln)
                for rb4 in range(4):
                    ob = ltpool.tile([128, VS], bf, tag="ob", bufs=4,
                                     name="ob")
                    for vq in range(4):
                        # balance the subtract across Pool (2) / DVE (2);
                        # ACT is the logits bottleneck, keep it clear
                        eng = nc.gpsimd if vq < 2 else nc.vector
                        eng.tensor_scalar_sub(
                            ob[:, 1024 * vq:1024 * (vq + 1)],
                            lbs[rb4][:, vq, :], lnS[:, rb4:rb4 + 1])
                    obs.append(ob)
                for rb4 in range(4):
                    rb = rbs[rb4]
                    nc.sync.dma_start(out_d[128 * rb:128 * (rb + 1), :],
                                      obs[rb4][:])

    nc.compile()
    return nc


def _get_nc():
    if "nc" not in _BUILT:
        _BUILT["nc"] = _build_nc()
    return _BUILT["nc"]


def _prep_inputs(inputs):
    """Host-side shard + relayout. Returns in_maps for 8 cores."""
    bft = ml_dtypes.bfloat16
    f8t = ml_dtypes.float8_e4m3

    tgt = np.asarray(inputs["target"])
    ctx = np.asarray(inputs["context"], np.float32)
    emb_t = np.asarray(inputs["embed_table"], np.float32)
    fc_w = np.asarray(inputs["fc_w"], np.float32)
    fc_b = np.asarray(inputs["fc_b"], np.float32)

    def wT(w, kc):     # [2, 1536, IN] -> [128, kc, 2, 12, 128]
        w = np.asarray(w, np.float32)
        a = w.transpose(2, 0, 1).reshape(kc, 128, 2, 12, 128)
        return np.ascontiguousarray(a.transpose(1, 0, 2, 3, 4)).astype(bft)

    WihT0 = wT(inputs["w_ih0"], 4)
    WhhT0 = wT(inputs["w_hh0"], 4)
    WihT1 = wT(inputs["w_ih1"], 8)
    WhhT1 = wT(inputs["w_hh1"], 4)

    # biasT[slot, layer, dir, g]
    biasT = np.zeros((16, 2, 2, 128), np.float32)
    for l, (bi, bh) in enumerate([
            (np.asarray(inputs["b_ih0"], np.float32),
             np.asarray(inputs["b_hh0"], np.float32)),
            (np.asarray(inputs["b_ih1"], np.float32),
             np.asarray(inputs["b_hh1"], np.float32))]):
        for d in range(2):
            rz = (bi[d, :1024] + bh[d, :1024]).reshape(8, 128)
            biasT[0:8, l, d, :] = rz
            biasT[8:12, l, d, :] = bh[d, 1024:].reshape(4, 128)
            biasT[12:16, l, d, :] = bi[d, 1024:].reshape(4, 128)
    biasT = biasT.astype(bft)

    sel = np.zeros((16, 16, W, BPC), np.float32)
    for s in range(16):
        sel[s, s] = 1.0
    sel = sel.astype(bft)

    fcw_pad = np.zeros((VPAD, 2 * H), np.float32)
    fcw_pad[:V] = fc_w
    fcb_pad = np.full((VPAD,), PADB, np.float32)
    fcb_pad[:V] = fc_b * (SW * SX)

    ones8 = np.zeros((1, 2, 128), np.float32)
    ones8[0, 0, :] = 1.0
    ones8 = ones8.astype(f8t)

    emb = emb_t[tgt]                      # [B, S, E]
    ctx4 = ctx.reshape(L, 2, B, H)        # [l, d, b, h]

    in_maps = []
    for c in range(NC_):
        bs = slice(BPC * c, BPC * (c + 1))
        emb_rows = emb[bs].transpose(1, 0, 2).reshape(R, E)   # row = 4t+b
        embT = np.ascontiguousarray(
            emb_rows.T.reshape(4, 128, R).transpose(1, 0, 2)).astype(bft)
        cc = ctx4[:, :, bs, :]                                # [l, d, 4, h]
        h0a = cc.transpose(3, 0, 1, 2).reshape(4, 128, L, 2, BPC)
        h0T = np.ascontiguousarray(
            h0a.transpose(1, 2, 3, 0, 4)).astype(bft)
        shard = fcw_pad[VS * c:VS * (c + 1)] * SW             # [VS, 1024]
        fcw8 = np.ascontiguousarray(
            shard.T.reshape(8, 128, VS).transpose(1, 0, 2)).astype(f8t)
        fcb8 = np.zeros((1, 4, 2, 1024), np.float32)
        fcb8[0, :, 0, :] = fcb_pad[VS * c:VS * (c + 1)].reshape(4, 1024)
        fcb8 = fcb8.astype(f8t)
        in_maps.append({
            "embT": embT, "h0T": h0T,
            "WihT0": WihT0, "WhhT0": WhhT0,
            "WihT1": WihT1, "WhhT1": WhhT1,
            "biasT": biasT, "sel16": sel,
            "fcw8": fcw8, "fcb8": fcb8, "ones8": ones8,
        })
    return in_maps


def _unshard(results):
    Lfull = np.concatenate(
        [results[c]["out"].astype(np.float32) for c in range(NC_)], axis=1)
    Lfull = Lfull[:, :V]                  # [4096, 32000]
    b = np.arange(B)[:, None]
    s = np.arange(S)[None, :]
    rows = (b // BPC) * R + BPC * s + (b % BPC)
    return Lfull[rows]                    # [B, S, V]


def kernel(**inputs):
    from concourse.bass_utils import run_bass_kernel_spmd
    nc = _get_nc()
    in_maps = _prep_inputs(inputs)
    res = run_bass_kernel_spmd(nc, in_maps, core_ids=list(range(NC_)))
    return _unshard(res.results)


# revision 42
# speedup vs baseline: 1.2560x; 1.0008x over previous
"""Trainium2 Bass kernel for nn_PlainDecoder (2-layer 2-dir GRU decoder +
vocab projection + log_softmax).

Sharding: data-parallel over batch (4 per core) for the scan; vocab-parallel
(4096-wide shard of padded 32768) for the logits.

Scan design (transposed orientation): all gate matmuls output
[128 gate-partitions, 4 batch] so the PE bill (prop. to output FREE size) is
tiny and h' is produced directly in lhsT (hidden-major) layout -- no PE
transposes.  Per (layer, dir) a windowed PSUM tile P holds, per step, 16
slots of 128 gates: [0:8]=r|z (preloaded with gi+bias), [8:12]=n-gh
(preloaded with b_hh_n), [12:16]=gi_n (+b_ih_n).  A bias matmul opens each
window bank (start=True), the windowed gi GEMM and the per-step Whh matmuls
accumulate on top (start=False).  Both layers run interleaved (layer 1 lags
LAG steps).  Everything bf16 into the PE, f32 in PSUM.

Logits: x2 (= layer-1 hist, bf16) is scaled to fp8 and AllGather'd; fc_w is
fp8.  Matmuls run in DoubleRow perf mode (K=256/instr, 0.5 cyc/row).  Per
(128-row block, 1024-vocab chunk): exp(l/256) with accumulated row sums and
a bf16 copy of l (frees PSUM fast); one AllReduce of partial sums per block;
out = l - ln(S) written bf16 (host converts to f32).
"""

import os
import sys
from contextlib import ExitStack

for _p in ("/opt/trn_rl_repo", "/root/.axon_site/_ro/trn_rl_repo"):
    if os.path.isdir(_p) and _p not in sys.path:
        sys.path.insert(0, _p)

import numpy as np  # noqa: E402
import ml_dtypes  # noqa: E402

V, E, H, L, B, S = 32000, 512, 512, 2, 32, 128
NC_ = 8                      # cores
BPC = B // NC_               # batches per core = 4
R = BPC * S                  # rows per core = 512 (s-major: row = 4*t + b)
VPAD = 32768
VS = VPAD // NC_             # vocab shard per core = 4096
W = 8                        # scan PSUM window (steps)
LAG = 11                     # layer-1 lag (steps)
NW = S // W                  # 16 windows
SW = 64.0                    # fc_w fp8 scale
SX = 4.0                     # x2 fp8 scale
SREC = 1.0 / (SW * SX)       # logits descale
PADB = -240.0                # pad-vocab scaled bias (e4m3 max finite)
NROW = NC_ * R               # 4096 global rows
NBLK = NROW // 128           # 32 row blocks

_BUILT = {}


def _build_nc(n_cores=NC_, sim=False):
    """Build the Bass program (same NEFF for all cores; per-core data
    differs).  sim=True replaces collectives with local DMAs so TimelineSim
    can run."""
    import concourse.bass as bass  # noqa: F401
    import concourse.mybir as mybir
    import concourse.tile as tile
    from concourse import bacc

    dt = mybir.dt
    f32 = dt.float32
    bf = dt.bfloat16
    f8 = dt.float8e4
    AF = mybir.ActivationFunctionType
    OP = mybir.AluOpType
    PM = mybir.MatmulPerfMode

    nc = bacc.Bacc("TRN2", target_bir_lowering=False, debug=False,
                   num_devices=n_cores)

    # ---------------- DRAM I/O ----------------
    embT = nc.dram_tensor("embT", [128, 4, R], bf, kind="ExternalInput")
    h0T = nc.dram_tensor("h0T", [128, 2, 2, 4, BPC], bf, kind="ExternalInput")
    WihT0 = nc.dram_tensor("WihT0", [128, 4, 2, 12, 128], bf,
                           kind="ExternalInput")
    WhhT0 = nc.dram_tensor("WhhT0", [128, 4, 2, 12, 128], bf,
                           kind="ExternalInput")
    WihT1 = nc.dram_tensor("WihT1", [128, 8, 2, 12, 128], bf,
                           kind="ExternalInput")
    WhhT1 = nc.dram_tensor("WhhT1", [128, 4, 2, 12, 128], bf,
                           kind="ExternalInput")
    biasT = nc.dram_tensor("biasT", [16, 2, 2, 128], bf, kind="ExternalInput")
    sel16 = nc.dram_tensor("sel16", [16, 16, W, BPC], bf, kind="ExternalInput")
    fcw8 = nc.dram_tensor("fcw8", [128, 8, VS], f8, kind="ExternalInput")
    fcb8 = nc.dram_tensor("fcb8", [1, 4, 2, 1024], f8, kind="ExternalInput")
    ones8 = nc.dram_tensor("ones8", [1, 2, 128], f8, kind="ExternalInput")

    out_d = nc.dram_tensor("out", [NROW, VS], bf, kind="ExternalOutput")

    # internal DRAM for collectives
    HR = R // 2
    agx_in = [nc.dram_tensor(f"agx_in{h}", [128, 8, HR], f8, kind="Internal")
              for h in range(2)]
    agx_out = [nc.dram_tensor(f"agx_out{h}", [n_cores * 128, 8, HR], f8,
                              kind="Internal", addr_space="Shared")
               for h in range(2)]
    NGRP = NBLK // 4          # 8 sum-collective groups of 4 row blocks
    ags_in = [nc.dram_tensor(f"ags_in{g}", [128, 4], f32, kind="Internal")
              for g in range(NGRP)]
    ags_out = [nc.dram_tensor(f"ags_out{g}", [128, 4], f32,
                              kind="Internal", addr_space="Shared")
               for g in range(NGRP)]
    rg = [list(range(n_cores))]

    with tile.TileContext(nc) as tc, ExitStack() as top:
        # logits weights pool (DMAs emitted after the scan weights so they
        # ride under the scan instead of delaying its start)
        fwpool = top.enter_context(tc.tile_pool(name="fw", bufs=1))
        fw = fwpool.tile([128, 8, VS], f8, tag="fw", name="fw")
        fb = fwpool.tile([1, 4, 2, 1024], f8, tag="fb", name="fb")
        on8 = fwpool.tile([1, 2, 128], f8, tag="on8", name="on8")

        # ---------------- scan phase ----------------
        with ExitStack() as scan_stack:
            wpool = scan_stack.enter_context(tc.tile_pool(name="wts", bufs=1))
            hpool = scan_stack.enter_context(tc.tile_pool(name="hist", bufs=1))
            cpool = scan_stack.enter_context(tc.tile_pool(name="chain", bufs=3))
            p0pool = scan_stack.enter_context(
                tc.tile_pool(name="P0", bufs=2, space="PSUM"))
            p1pool = scan_stack.enter_context(
                tc.tile_pool(name="P1", bufs=2, space="PSUM"))

            embT_sb = wpool.tile([128, 4, R], bf, tag="embT", name="embT_sb")
            nc.sync.dma_start(embT_sb[:], embT[:])
            h0_sb = wpool.tile([128, 2, 2, 4, BPC], bf, tag="h0", name="h0_sb")
            nc.sync.dma_start(h0_sb[:], h0T[:])
            wih0 = wpool.tile([128, 4, 2, 12, 128], bf, tag="wih0",
                              name="wih0")
            nc.sync.dma_start(wih0[:], WihT0[:])
            whh0 = wpool.tile([128, 4, 2, 12, 128], bf, tag="whh0",
                              name="whh0")
            nc.sync.dma_start(whh0[:], WhhT0[:])
            wih1 = wpool.tile([128, 8, 2, 12, 128], bf, tag="wih1",
                              name="wih1")
            nc.sync.dma_start(wih1[:], WihT1[:])
            whh1 = wpool.tile([128, 4, 2, 12, 128], bf, tag="whh1",
                              name="whh1")
            nc.sync.dma_start(whh1[:], WhhT1[:])
            bias_sb = wpool.tile([16, 2, 2, 128], bf, tag="bias",
                                 name="bias_sb")
            nc.sync.dma_start(bias_sb[:], biasT[:])
            sel_sb = wpool.tile([16, 16, W, BPC], bf, tag="sel", name="sel_sb")
            nc.sync.dma_start(sel_sb[:], sel16[:])
            nc.sync.dma_start(fw[:], fcw8[:])
            nc.sync.dma_start(fb[:], fcb8[:])
            nc.sync.dma_start(on8[:], ones8[:])

            # hist layout: [128 h-part, dir, kchunk, row(=4t+b)]
            hist = [hpool.tile([128, 2, 4, R], bf, tag=f"hist{l}",
                               name=f"hist{l}") for l in range(2)]
            ppool = [p0pool, p1pool]
            wih = [wih0, wih1]
            whh = [whh0, whh1]
            kc_ih = [4, 8]
            pwin = [{}, {}]       # (layer, window) -> PSUM tile
            gwin = [{}, {}]       # (layer, window) -> SBUF gi_n tile

            def gi_window(l, w, dirs=(0, 1)):
                """Bias opener + gi GEMM for window w of layer l (emitted
                per dir so late windows can be sliced across iterations).
                P layout: [128, dir, slot16, W, BPC]."""
                if 0 in dirs:
                    P = ppool[l].tile([128, 2, 16, W, BPC], f32, tag=f"P{l}",
                                      name=f"P{l}w{w}")
                    pwin[l][w] = P
                    gin = cpool.tile([128, 2, 4, W, BPC], f32, tag=f"gin{l}",
                                     bufs=2, name=f"gin{l}")
                    gwin[l][w] = gin
                P = pwin[l][w]
                gin = gwin[l][w]
                rows = slice(BPC * W * w, BPC * W * (w + 1))
                for d in dirs:
                    nc.tensor.matmul(P[:, d], bias_sb[:, l, d, :], sel_sb[:],
                                     start=True, stop=False,
                                     skip_group_check=True)
                    for sl in range(12):
                        slot = sl if sl < 8 else sl + 4
                        for k in range(kc_ih[l]):
                            if l == 0:
                                rhs = embT_sb[:, k, rows]
                            else:
                                rhs = hist[0][:, k // 4, k % 4, rows]
                            nc.tensor.matmul(
                                P[:, d, slot], wih[l][:, k, d, sl, :], rhs,
                                start=False,
                                stop=(slot >= 12 and k == kc_ih[l] - 1),
                                skip_group_check=True)
                    # stage gi_n into SBUF so the in-loop add is SBUF-only
                    nc.vector.tensor_copy(gin[:, d], P[:, d, 12:16, :, :])

            def scan_step(l, t):
                """Whh matmuls + GRU cell chain for step t of layer l."""
                P = pwin[l][t // W]
                gin = gwin[l][t // W]
                tw = t % W
                if t == 0:
                    hp = h0_sb[:, l]                      # [128, 2, 4, BPC]
                else:
                    hp = hist[l][:, :, :, BPC * (t - 1):BPC * t]
                # k-major, r|z slices first: k0/k1 matmuls wait only on the
                # first half of the h' tail, and the sigmoid (head of the
                # serial chain) doesn't wait on the n-gh matmuls
                for j0, j1 in ((0, 8), (8, 12)):
                    for k in range(4):
                        for d in range(2):
                            for j in range(j0, j1):
                                nc.tensor.matmul(
                                    P[:, d, j, tw, :], whh[l][:, k, d, j, :],
                                    hp[:, d, k, :], start=False,
                                    stop=(k == 3), skip_group_check=True)
                # GRU cell chain (both dirs in one op each):
                #   h' = n*(1-z) + z*hprev ; q=z*hprev and u=1-z leave the
                #   serial path right after the sigmoid
                rzs = cpool.tile([128, 2, 8, BPC], f32, tag=f"rzs{l}",
                                 name=f"rzs{l}")
                nc.scalar.activation(rzs[:], P[:, :, 0:8, tw, :], AF.Sigmoid)
                n1 = cpool.tile([128, 2, 4, BPC], f32, tag=f"n1{l}",
                                name=f"n1{l}")
                nc.vector.tensor_mul(n1[:], P[:, :, 8:12, tw, :],
                                     rzs[:, :, 0:4, :])
                nc.vector.tensor_add(n1[:], n1[:], gin[:, :, :, tw, :])
                q = cpool.tile([128, 2, 4, BPC], f32, tag=f"q{l}",
                               name=f"q{l}")
                nc.vector.tensor_mul(q[:], rzs[:, :, 4:8, :], hp[:])
                u = cpool.tile([128, 2, 4, BPC], f32, tag=f"u{l}",
                               name=f"u{l}")
                nc.vector.tensor_scalar(u[:], rzs[:, :, 4:8, :], -1.0, 1.0,
                                        OP.mult, OP.add)
                nt = cpool.tile([128, 2, 4, BPC], f32, tag=f"nt{l}",
                                name=f"nt{l}")
                nc.scalar.activation(nt[:], n1[:], AF.Tanh)
                # tail on Pool (no ack latency, SBUF-only operands), split per
                # k-half so next-step k0/k1 matmuls start before k2/k3 finish
                d1 = cpool.tile([128, 2, 4, BPC], f32, tag=f"d1{l}",
                                name=f"d1{l}")
                nc.gpsimd.tensor_mul(d1[:], nt[:], u[:])
                nc.gpsimd.tensor_add(
                    hist[l][:, :, :, BPC * t:BPC * (t + 1)], d1[:], q[:])

            def ship_x2(h):
                rows = slice(HR * h, HR * (h + 1))
                x8 = cpool.tile([128, 2, 4, HR], f8, tag="x8", bufs=2,
                                name="x8")
                nc.vector.tensor_scalar_mul(x8[:], hist[1][:, :, :, rows], SX)
                nc.sync.dma_start(agx_in[h][:],
                                  x8[:].rearrange("p d k r -> p (d k) r"))
                if sim:
                    nc.sync.dma_start(agx_out[h][0:128], agx_in[h][:])
                else:
                    nc.gpsimd.collective_compute(
                        "AllGather", OP.bypass, replica_groups=rg,
                        ins=[agx_in[h][:].opt()], outs=[agx_out[h][:].opt()])

            gi_window(0, 0)
            for it in range(S + LAG):
                if it == S // 2 + LAG + 1:
                    ship_x2(0)        # L1 rows 0..255 done; gather them now
                if it % W == 5 and (it + 3) // W < NW:
                    gi_window(0, (it + 3) // W, dirs=(0,))
                if it % W == 6 and (it + 2) // W < NW:
                    gi_window(0, (it + 2) // W, dirs=(1,))
                if it % W == 1 and it >= 9 and (it - 9) // W < NW:
                    gi_window(1, (it - 9) // W, dirs=(0,))
                if it % W == 2 and it >= 10 and (it - 10) // W < NW:
                    gi_window(1, (it - 10) // W, dirs=(1,))
                if it < S:
                    scan_step(0, it)
                t1 = it - LAG
                if 0 <= t1 < S:
                    scan_step(1, t1)

            ship_x2(1)

        # ---------------- logits + log_softmax ----------------
        with (
            tc.tile_pool(name="lt", bufs=3) as ltpool,
            tc.tile_pool(name="lps", bufs=4, space="PSUM") as lpspool,
            tc.tile_pool(name="lsc", bufs=2) as lscpool,
        ):
            x2g = ltpool.tile([128, 8, 8, R], f8, tag="x2g", bufs=1,
                              name="x2g")
            # per-source-core, per-half chunks: contiguous DMAs; the first
            # half is gathered mid-scan, so early row blocks start sooner
            for c in range(8):
                for h in range(2):
                    nc.sync.dma_start(
                        x2g[:, :, c, HR * h:HR * (h + 1)],
                        agx_out[h][128 * c:128 * (c + 1), :, :])

            def block_mm(rb, lb, sgrp, rb4):
                """Matmuls + exp/sums + bf16-l copy for one 128-row block.
                All 40 matmuls are emitted back-to-back (keeps the PE
                p-state ramped); exp/copies follow."""
                csrc, r0 = rb // BPC, (rb % BPC) * 128
                srb = lscpool.tile([128, 4], f32, tag="srb", bufs=3,
                                   name="srb")
                def vq_mms(vq):
                    P = lpspool.tile([128, 1024], f32, tag="lp", name="lp")
                    # matmuls per 512-wide half: a PSUM matmul target must
                    # stay within one 2KB bank
                    for hh in range(2):
                        v0 = 1024 * vq + 512 * hh
                        nc.tensor.matmul(P[:, 512 * hh:512 * (hh + 1)],
                                         on8[:], fb[0:1, vq, :,
                                                    512 * hh:512 * (hh + 1)],
                                         start=True, stop=False,
                                         perf_mode=PM.DoubleRow,
                                         skip_group_check=True)
                        for c2 in range(4):
                            nc.tensor.matmul(
                                P[:, 512 * hh:512 * (hh + 1)],
                                x2g[:, 2 * c2:2 * c2 + 2, csrc, r0:r0 + 128],
                                fw[:, 2 * c2:2 * c2 + 2, v0:v0 + 512],
                                start=False, stop=(c2 == 3),
                                perf_mode=PM.DoubleRow, skip_group_check=True)
                    return P

                def vq_post(vq, P):
                    eb = lscpool.tile([128, 1024], bf, tag="eb", bufs=3,
                                      name="eb")
                    nc.scalar.activation(eb[:], P[:], AF.Exp, scale=SREC,
                                         accum_out=srb[:, vq:vq + 1])
                    # bf16 copy of l frees the PSUM bank quickly
                    nc.vector.tensor_scalar_mul(lb[:, vq, :], P[:], SREC)

                # software-pipelined: chunk vq's exp/copy emitted after chunk
                # vq+1's matmuls so the PE stream never waits on PSUM reuse
                Ps = [vq_mms(0)]
                for vq in range(1, 4):
                    Ps.append(vq_mms(vq))
                    vq_post(vq - 1, Ps[vq - 1])
                vq_post(3, Ps[3])
                nc.vector.tensor_reduce(sgrp[:, rb4:rb4 + 1], srb[:],
                                        axis=mybir.AxisListType.X, op=OP.add)

            for g in range(NGRP):
                rbs = list(range(4 * g, 4 * g + 4))
                lbs = []
                obs = []
                sgrp = lscpool.tile([128, 4], f32, tag="sgrp", bufs=3,
                                    name="sgrp")
                for rb4 in range(4):
                    lb = ltpool.tile([128, 4, 1024], bf, tag="lb", bufs=6,
                                     name="lb")
                    block_mm(rbs[rb4], lb, sgrp, rb4)
                    lbs.append(lb)
                nc.sync.dma_start(ags_in[g][:], sgrp[:])
                if sim:
                    nc.sync.dma_start(ags_out[g][:], ags_in[g][:])
                else:
                    nc.gpsimd.collective_compute(
                        "AllReduce", OP.add, replica_groups=rg,
                        ins=[ags_in[g][:].opt()],
                        outs=[ags_out[g][:].opt()])
                sg = lscpool.tile([128, 4], f32, tag="sg", name="sg")
                nc.sync.dma_start(sg[:], ags_out[g][:])
                lnS = lscpool.tile([128, 4], f32, tag="lnS", name="lnS")
                nc.scalar.activation(lnS[:], sg[:], AF.Ln)
                for rb4 in range(4):
                    ob = ltpool.tile([128, VS], bf, tag="ob", bufs=4,
                                     name="ob")
                    for vq in range(4):
                        # balance the subtract across Pool (2) / DVE (2);
                        # ACT is the logits bottleneck, keep it clear
                        eng = nc.gpsimd if vq < 2 else nc.vector
                        eng.tensor_scalar_sub(
                            ob[:, 1024 * vq:1024 * (vq + 1)],
                            lbs[rb4][:, vq, :], lnS[:, rb4:rb4 + 1])
                    obs.append(ob)
                for rb4 in range(4):
                    rb = rbs[rb4]
                    nc.sync.dma_start(out_d[128 * rb:128 * (rb + 1), :],
                                      obs[rb4][:])

    nc.compile()
    return nc


def _get_nc():
    if "nc" not in _BUILT:
        _BUILT["nc"] = _build_nc()
    return _BUILT["nc"]


def _prep_inputs(inputs):
    """Host-side shard + relayout. Returns in_maps for 8 cores."""
    bft = ml_dtypes.bfloat16
    f8t = ml_dtypes.float8_e4m3

    tgt = np.asarray(inputs["target"])
    ctx = np.asarray(inputs["context"], np.float32)
    emb_t = np.asarray(inputs["embed_table"], np.float32)
    fc_w = np.asarray(inputs["fc_w"], np.float32)
    fc_b = np.asarray(inputs["fc_b"], np.float32)

    def wT(w, kc):     # [2, 1536, IN] -> [128, kc, 2, 12, 128]
        w = np.asarray(w, np.float32)
        a = w.transpose(2, 0, 1).reshape(kc, 128, 2, 12, 128)
        return np.ascontiguousarray(a.transpose(1, 0, 2, 3, 4)).astype(bft)

    WihT0 = wT(inputs["w_ih0"], 4)
    WhhT0 = wT(inputs["w_hh0"], 4)
    WihT1 = wT(inputs["w_ih1"], 8)
    WhhT1 = wT(inputs["w_hh1"], 4)

    # biasT[slot, layer, dir, g]
    biasT = np.zeros((16, 2, 2, 128), np.float32)
    for l, (bi, bh) in enumerate([
            (np.asarray(inputs["b_ih0"], np.float32),
             np.asarray(inputs["b_hh0"], np.float32)),
            (np.asarray(inputs["b_ih1"], np.float32),
             np.asarray(inputs["b_hh1"], np.float32))]):
        for d in range(2):
            rz = (bi[d, :1024] + bh[d, :1024]).reshape(8, 128)
            biasT[0:8, l, d, :] = rz
            biasT[8:12, l, d, :] = bh[d, 1024:].reshape(4, 128)
            biasT[12:16, l, d, :] = bi[d, 1024:].reshape(4, 128)
    biasT = biasT.astype(bft)

    sel = np.zeros((16, 16, W, BPC), np.float32)
    for s in range(16):
        sel[s, s] = 1.0
    sel = sel.astype(bft)

    fcw_pad = np.zeros((VPAD, 2 * H), np.float32)
    fcw_pad[:V] = fc_w
    fcb_pad = np.full((VPAD,), PADB, np.float32)
    fcb_pad[:V] = fc_b * (SW * SX)

    ones8 = np.zeros((1, 2, 128), np.float32)
    ones8[0, 0, :] = 1.0
    ones8 = ones8.astype(f8t)

    emb = emb_t[tgt]                      # [B, S, E]
    ctx4 = ctx.reshape(L, 2, B, H)        # [l, d, b, h]

    in_maps = []
    for c in range(NC_):
        bs = slice(BPC * c, BPC * (c + 1))
        emb_rows = emb[bs].transpose(1, 0, 2).reshape(R, E)   # row = 4t+b
        embT = np.ascontiguousarray(
            emb_rows.T.reshape(4, 128, R).transpose(1, 0, 2)).astype(bft)
        cc = ctx4[:, :, bs, :]                                # [l, d, 4, h]
        h0a = cc.transpose(3, 0, 1, 2).reshape(4, 128, L, 2, BPC)
        h0T = np.ascontiguousarray(
            h0a.transpose(1, 2, 3, 0, 4)).astype(bft)
        shard = fcw_pad[VS * c:VS * (c + 1)] * SW             # [VS, 1024]
        fcw8 = np.ascontiguousarray(
            shard.T.reshape(8, 128, VS).transpose(1, 0, 2)).astype(f8t)
        fcb8 = np.zeros((1, 4, 2, 1024), np.float32)
        fcb8[0, :, 0, :] = fcb_pad[VS * c:VS * (c + 1)].reshape(4, 1024)
        fcb8 = fcb8.astype(f8t)
        in_maps.append({
            "embT": embT, "h0T": h0T,
            "WihT0": WihT0, "WhhT0": WhhT0,
            "WihT1": WihT1, "WhhT1": WhhT1,
            "biasT": biasT, "sel16": sel,
            "fcw8": fcw8, "fcb8": fcb8, "ones8": ones8,
        })
    return in_maps


def _unshard(results):
    Lfull = np.concatenate(
        [results[c]["out"].astype(np.float32) for c in range(NC_)], axis=1)
    Lfull = Lfull[:, :V]                  # [4096, 32000]
    b = np.arange(B)[:, None]
    s = np.arange(S)[None, :]
    rows = (b // BPC) * R + BPC * s + (b % BPC)
    return Lfull[rows]                    # [B, S, V]


def kernel(**inputs):
    from concourse.bass_utils import run_bass_kernel_spmd
    nc = _get_nc()
    in_maps = _prep_inputs(inputs)
    res = run_bass_kernel_spmd(nc, in_maps, core_ids=list(range(NC_)))
    return _unshard(res.results)
